# revision 21
# baseline (speedup 1.0000x reference)
"""DTCWT 3-level inverse on 8 Trainium2 NeuronCores.

Every filtering stage is a banded matmul on the tensor engine in fp16
(PSUM accumulates fp32; ~7e-4 total rel err vs the 2e-2 gate).

All stages use "data as lhsT" mode: matmul(out, lhsT=data[K=h, M=w],
rhs=mat[K=h, N=h_out]) contracts over the partition dim of the data and
yields the filtered image TRANSPOSED ([w, h_out]); column and row stages
then alternate orientation naturally with zero explicit transposes.

The c2q band construction is folded into the matrices; at L1 the lowpass
path is additionally merged into the band polyphase layout ([E|O] w-planes)
so the final row stage is 4 accumulation passes instead of 6.

Schedule: phase-major (L3 x16, L2 x16, L1 x16) with double/triple-buffered
PSUM pools so neighbouring images' matmuls hide each other's copy latency.
DMA queue slots cost ~600ns regardless of size, so all loads are batched
into a handful of giant multi-dim DMAs (2 matrix blobs, 9 input sweeps,
1 store per image).

Sharding: pure data parallel over batch N (8 cores x 16 channels each).
"""
import sys

for _p in ('/opt/trn_rl_repo',):
    if _p not in sys.path:
        sys.path.append(_p)

import numpy as np
import concourse.bass as bass
import concourse.mybir as mybir
from concourse.tile import TileContext
from concourse.bass_utils import run_bass_kernel_spmd

SQRT_HALF = 0.7071067811865476
N_CORES = 8
IMGS_PER_CORE = 16
F32 = mybir.dt.float32
F16 = mybir.dt.float16


# ---------------------------------------------------------------------------
# Host-side matrix construction (numpy, float64)
# ---------------------------------------------------------------------------
def _conv_rows_valid(x, h):
    hr = h[::-1]
    taps = h.shape[0]
    n = x.shape[-2] - taps + 1
    out = hr[0] * x[..., 0:n, :]
    for k in range(1, taps):
        out = out + hr[k] * x[..., k:k + n, :]
    return out


def _pad_rows_symmetric(x, m):
    pad = [(0, 0)] * (x.ndim - 2) + [(m, m), (0, 0)]
    return np.pad(x, pad, mode='symmetric')


def _colfilter(x, h):
    return _conv_rows_valid(_pad_rows_symmetric(x, h.shape[0] // 2), h)


def _colifilt(x, ha, hb, highpass):
    m = ha.shape[0]
    m2 = m // 2
    r = x.shape[-2]
    xp = _pad_rows_symmetric(x, m2)
    xe = xp[..., 1:r + m - 2:2, :]
    xo = xp[..., 2:r + m - 1:2, :]
    xa, xb = (xe, xo) if highpass else (xo, xe)
    hao, hae = ha[0::2], ha[1::2]
    hbo, hbe = hb[0::2], hb[1::2]
    y0 = _conv_rows_valid(xb, hao)
    y1 = _conv_rows_valid(xa, hbo)
    y2 = _conv_rows_valid(xb, hae)
    y3 = _conv_rows_valid(xa, hbe)
    y = np.stack([y0, y1, y2, y3], axis=-2)
    return y.reshape(y.shape[:-3] + (2 * r, y.shape[-1]))


def _op_matrix(op, n):
    """M[h_in, h_out] with out[h_out, w] = sum_h M[h, h_out] x[h, w]."""
    return np.ascontiguousarray(op(np.eye(n, dtype=np.float64)).T)


def build_matrices(g0o, g1o, g0a, g0b, g1a, g1b):
    """All device matrices as {name: fp16 ndarray}."""
    g0o = np.asarray(g0o, np.float64)
    g1o = np.asarray(g1o, np.float64)
    g0a = np.asarray(g0a, np.float64)
    g0b = np.asarray(g0b, np.float64)
    g1a = np.asarray(g1a, np.float64)
    g1b = np.asarray(g1b, np.float64)
    s = SQRT_HALF
    hs, vs = np.hstack, np.vstack
    out = {}

    def upsample_level(R, tag):
        Mlo = _op_matrix(lambda x: _colifilt(x, g0b, g0a, False), R)  # [R, 2R]
        Mhi = _op_matrix(lambda x: _colifilt(x, g1b, g1a, True), R)
        Me_h, Mo_h = s * Mhi[0::2], s * Mhi[1::2]                     # [R/2, 2R]
        Me_l, Mo_l = s * Mlo[0::2], s * Mlo[1::2]
        out[f'M{tag}_lo'] = Mlo
        # pair-stacked [w1; w2] col rhs, e|o column-concatenated
        #   e: w1r*Me + w2r*Me + w1i*Mo - w2i*Mo
        #   o: -w1r*Mo + w2r*Mo + w1i*Me + w2i*Me
        out[f'L{tag}_hi_R'] = hs([vs([Me_h, Me_h]), vs([-Mo_h, Mo_h])])
        out[f'L{tag}_hi_I'] = hs([vs([Mo_h, -Mo_h]), vs([Me_h, Me_h])])
        out[f'L{tag}_lo_R'] = hs([vs([Me_l, Me_l]), vs([-Mo_l, Mo_l])])
        out[f'L{tag}_lo_I'] = hs([vs([Mo_l, -Mo_l]), vs([Me_l, Me_l])])
        # row stage (polyphase-column recombination)
        out[f'Be{tag}_lo'], out[f'Bo{tag}_lo'] = Mlo[0::2], Mlo[1::2]
        out[f'Be{tag}_hi'], out[f'Bo{tag}_hi'] = Mhi[0::2], Mhi[1::2]

    upsample_level(64, '3')
    upsample_level(128, '2')
    # L3 quad stacks: [hl pair (lo mats); hh pair (hi mats)], K=128
    out['L3_q_R'] = vs([out['L3_lo_R'], out['L3_hi_R']])
    out['L3_q_I'] = vs([out['L3_lo_I'], out['L3_hi_I']])
    del out['L3_lo_R'], out['L3_lo_I']  # only used inside the quad at L3
    # K-stacked row-stage rhs at L2 (lhsT pieces partition-stacked)
    out['R2_E'] = vs([out['Be2_lo'], out['Be2_hi']])
    out['R2_O'] = vs([out['Bo2_lo'], out['Bo2_hi']])
    for k in ('Be2_lo', 'Bo2_lo', 'Be2_hi', 'Bo2_hi'):
        del out[k]

    # L1 (colfilter, size-preserving, n=256)
    A_lo = _op_matrix(lambda x: _colfilter(x, g0o), 256)              # [256, 256]
    A_hi = _op_matrix(lambda x: _colfilter(x, g1o), 256)
    out['Alo_a'], out['Alo_b'] = A_lo[0:128], A_lo[128:256]
    for x, A in (('hi', A_hi), ('lo', A_lo)):
        Me, Mo = s * A[0::2], s * A[1::2]                             # [128, 256]
        out[f'L1{x}_w1r'] = hs([Me, -Mo])
        out[f'L1{x}_w2r'] = hs([Me, Mo])
        out[f'L1{x}_w1i'] = hs([Mo, Me])
        out[f'L1{x}_w2i'] = hs([-Mo, Me])
    out['Be1_lo'], out['Bo1_lo'] = A_lo[0::2], A_lo[1::2]
    out['Be1_hi'], out['Bo1_hi'] = A_hi[0::2], A_hi[1::2]
    # zero-top variants: lhsT base partitions are limited to {0,32,64}, so
    # the 4th 32-row band slot (base 96) runs as K=64 at base 64 with the
    # top half of the matrix zeroed.
    z32 = np.zeros((32, 128))
    for nm in ('Be3_lo', 'Bo3_lo', 'Be3_hi', 'Bo3_hi'):
        out[nm + 'Z'] = np.vstack([z32, out[nm]])
    return {k: np.ascontiguousarray(v, np.float16) for k, v in out.items()}


MAT_SHAPES = {
    'M3_lo': (64, 128),
    'L3_hi_R': (64, 256), 'L3_hi_I': (64, 256),
    'L3_q_R': (128, 256), 'L3_q_I': (128, 256),
    'Be3_lo': (32, 128), 'Bo3_lo': (32, 128),
    'Be3_hi': (32, 128), 'Bo3_hi': (32, 128),
    'Be3_loZ': (64, 128), 'Bo3_loZ': (64, 128),
    'Be3_hiZ': (64, 128), 'Bo3_hiZ': (64, 128),
    'M2_lo': (128, 256),
    'L2_hi_R': (128, 512), 'L2_hi_I': (128, 512),
    'L2_lo_R': (128, 512), 'L2_lo_I': (128, 512),
    'R2_E': (128, 256), 'R2_O': (128, 256),
    'Alo_a': (128, 256), 'Alo_b': (128, 256),
    'L1hi_w1r': (128, 512), 'L1hi_w2r': (128, 512),
    'L1hi_w1i': (128, 512), 'L1hi_w2i': (128, 512),
    'L1lo_w1r': (128, 512), 'L1lo_w2r': (128, 512),
    'L1lo_w1i': (128, 512), 'L1lo_w2i': (128, 512),
    'Be1_lo': (128, 256), 'Bo1_lo': (128, 256),
    'Be1_hi': (128, 256), 'Bo1_hi': (128, 256),
}

BLOB_A = ['M3_lo', 'L3_hi_R', 'L3_hi_I', 'L3_q_R', 'L3_q_I',
          'Be3_lo', 'Bo3_lo', 'Be3_hi', 'Bo3_hi',
          'Be3_loZ', 'Bo3_loZ', 'Be3_hiZ', 'Bo3_hiZ',
          'M2_lo', 'L2_hi_R', 'L2_hi_I', 'L2_lo_R', 'L2_lo_I',
          'R2_E', 'R2_O']
BLOB_B = ['Alo_a', 'Alo_b',
          'L1hi_w1r', 'L1hi_w2r', 'L1hi_w1i', 'L1hi_w2i',
          'L1lo_w1r', 'L1lo_w2r', 'L1lo_w1i', 'L1lo_w2i',
          'Be1_lo', 'Bo1_lo', 'Be1_hi', 'Bo1_hi']
BLOB_A_COLS = sum(MAT_SHAPES[n][1] for n in BLOB_A)
BLOB_B_COLS = sum(MAT_SHAPES[n][1] for n in BLOB_B)


def pack_blobs(mats):
    def pack(names, cols):
        blob = np.zeros((128, cols), np.float16)
        c = 0
        for n in names:
            K, N = MAT_SHAPES[n]
            for r in range(128 // K):  # replicate K<128 mats across parts
                blob[r * K:(r + 1) * K, c:c + N] = mats[n]
            c += N
        return blob
    return pack(BLOB_A, BLOB_A_COLS), pack(BLOB_B, BLOB_B_COLS)


# ---------------------------------------------------------------------------
# Bass kernel
# ---------------------------------------------------------------------------
def split_excess_waits(nc, max_waits=1):
    """walrus CTRL codegen allows only one sem wait per instruction; move
    excess waits onto NoOps inserted just before the offending instruction."""
    ctr = 0
    for fn in nc.m.functions:
        for bb in fn.blocks:
            new_list = []
            for inst in bb.instructions:
                si = inst.sync_info
                if si is not None and si.on_wait and len(si.on_wait) > max_waits:
                    waits = list(si.on_wait)
                    keep, extra = waits[:max_waits], waits[max_waits:]
                    for i in range(0, len(extra), max_waits):
                        nop = mybir.InstNoOp(
                            name=f"wait_split_{ctr}", ins=[], outs=[])
                        ctr += 1
                        nop.engine = inst.engine
                        nop.sync_info = mybir.SyncInfo(
                            on_wait=extra[i:i + max_waits], on_update=[])
                        nc.register_instruction(nop)
                        new_list.append(nop)
                    inst.sync_info = mybir.SyncInfo(
                        on_wait=keep,
                        on_update=list(si.on_update) if si.on_update else [])
                new_list.append(inst)
            bb.instructions[:] = new_list
    return ctr


def build_nc():
    nc = bass.Bass()
    yl_d = nc.dram_tensor("yl", [IMGS_PER_CORE, 64, 64], F16,
                          kind="ExternalInput")
    yh2_d = nc.dram_tensor("yh2", [IMGS_PER_CORE, 6, 32, 32, 2], F16,
                           kind="ExternalInput")
    yh1_d = nc.dram_tensor("yh1", [IMGS_PER_CORE, 6, 64, 64, 2], F16,
                           kind="ExternalInput")
    yh0_d = nc.dram_tensor("yh0", [IMGS_PER_CORE, 6, 128, 128, 2], F16,
                           kind="ExternalInput")
    out_d = nc.dram_tensor("out", [IMGS_PER_CORE, 256, 256], F16,
                           kind="ExternalOutput")
    matsA_d = nc.dram_tensor("matsA", [128, BLOB_A_COLS], F16,
                             kind="ExternalInput")
    matsB_d = nc.dram_tensor("matsB", [128, BLOB_B_COLS], F16,
                             kind="ExternalInput")

    with TileContext(nc) as tc:
        with tc.tile_pool(name="mats", bufs=1) as matpool, \
             tc.tile_pool(name="ins", bufs=1) as inpool, \
             tc.tile_pool(name="zs", bufs=1) as zpool, \
             tc.tile_pool(name="mid", bufs=3) as midpool, \
             tc.tile_pool(name="outp", bufs=4) as outpool:

            # --- matrix blobs: L3 head first so img0 can start early ---
            L3_HEAD = 2176  # cols of the 9 L3 matrices at the blobA front
            blobA_t = matpool.tile([128, BLOB_A_COLS], F16, tag="blobA")
            nc.scalar.dma_start(out=blobA_t[:, 0:L3_HEAD],
                                in_=matsA_d[:, 0:L3_HEAD])
            nc.scalar.dma_start(out=blobA_t[:, L3_HEAD:],
                                in_=matsA_d[:, L3_HEAD:])
            blobB_t = matpool.tile([128, BLOB_B_COLS], F16, tag="blobB")
            nc.sync.dma_start(out=blobB_t[:], in_=matsB_d[:])
            mats = {}
            mat_loc = {}
            cA = cB = 0
            for n in BLOB_A:
                K, N = MAT_SHAPES[n]
                mats[n] = blobA_t[0:K, cA:cA + N]
                mat_loc[n] = (blobA_t, cA)
                cA += N
            for n in BLOB_B:
                K, N = MAT_SHAPES[n]
                mats[n] = blobB_t[0:K, cB:cB + N]
                mat_loc[n] = (blobB_t, cB)
                cB += N

            def mat_at(name, poff):
                blob, c = mat_loc[name]
                K, N = MAT_SHAPES[name]
                return blob[poff:poff + K, c:c + N]

            # --- batched input sweeps ---
            # z3all[h, (i w)] <- yl[i, h, w]
            z3all = inpool.tile([64, 16 * 64], F16, tag="z3all")
            nc.gpsimd.dma_start(
                out=z3all.rearrange("h (i x) -> h i x", i=16),
                in_=yl_d.rearrange("i h x -> h i x"))
            # per-orientation all-image sweeps (DMA APs max 3 dims)
            def band_sweep(tile_ap, p0, p1, src5, i=16):
                nc.gpsimd.dma_start(
                    out=tile_ap[p0:p1, :].rearrange("h (i x) -> h i x", i=i),
                    in_=src5.rearrange("i h w r -> h i (w r)"))
            lh3all = inpool.tile([64, 16 * 64], F16, tag="lh3all")
            band_sweep(lh3all, 0, 32, yh2_d[:, 0])
            band_sweep(lh3all, 32, 64, yh2_d[:, 5])
            # q3all: parts 0:64 = orient pair (2,3), 64:128 = (1,4)
            q3all = inpool.tile([128, 16 * 64], F16, tag="q3all")
            band_sweep(q3all, 0, 32, yh2_d[:, 2])
            band_sweep(q3all, 32, 64, yh2_d[:, 3])
            band_sweep(q3all, 64, 96, yh2_d[:, 1])
            band_sweep(q3all, 96, 128, yh2_d[:, 4])
            # yh1 band pair tiles, one DMA per orientation
            lh2all = inpool.tile([128, 16 * 128], F16, tag="lh2all")
            band_sweep(lh2all, 0, 64, yh1_d[:, 0])
            band_sweep(lh2all, 64, 128, yh1_d[:, 5])
            hl2all = inpool.tile([128, 16 * 128], F16, tag="hl2all")
            band_sweep(hl2all, 0, 64, yh1_d[:, 2])
            band_sweep(hl2all, 64, 128, yh1_d[:, 3])
            hh2all = inpool.tile([128, 16 * 128], F16, tag="hh2all")
            band_sweep(hh2all, 0, 64, yh1_d[:, 1])
            band_sweep(hh2all, 64, 128, yh1_d[:, 4])
            # yh0: 4 groups of 4 imgs on the sync queue
            yh0all = inpool.tile([128, 16 * 1536], F16, tag="yh0all")
            for g in range(4):
                nc.sync.dma_start(
                    out=yh0all[:, g * 6144:(g + 1) * 6144].rearrange(
                        "h (g x) -> h g x", g=24),
                    in_=yh0_d[4 * g:4 * g + 4].rearrange(
                        "i o h w r -> h (i o) (w r)"))

            z2s = {img: zpool.tile([128, 128], F16, tag=f"z2_{img}",
                                   name=f"z2_{img}")
                   for img in range(IMGS_PER_CORE)}
            z1s = {img: zpool.tile([128, 512], F16, tag=f"z1_{img}",
                                   name=f"z1_{img}")
                   for img in range(IMGS_PER_CORE)}

            def mm(out_ap, lhsT, rhs_name, start, stop, poff=0):
                rhs = mats[rhs_name] if poff == 0 else mat_at(rhs_name, poff)
                nc.tensor.matmul(out_ap, lhsT, rhs, start=start, stop=stop)

            # ===========================================================
            # Phase L3: quad-packed (M=128 across 4 imgs) col stages,
            # per-img row stages via lhsT partition offsets
            # ===========================================================
            with tc.tile_pool(name="ps3c", bufs=2, space="PSUM") as ps3cpool,\
                 tc.tile_pool(name="ps3r", bufs=2, space="PSUM") as ps3rpool:
                for g in range(4):
                    base = 4 * g
                    c0 = base * 64
                    p3 = ps3cpool.tile([128, 1024], F32, tag="p3")
                    mm(p3[:, 0:128], z3all[:, c0:c0 + 128], 'M3_lo',
                       True, True)
                    mm(p3[:, 128:256], z3all[:, c0 + 128:c0 + 256], 'M3_lo',
                       True, True)
                    lq = lh3all[:, c0:c0 + 256]
                    mm(p3[:, 256:512], lq[:, 0::2], 'L3_hi_R', True, False)
                    mm(p3[:, 256:512], lq[:, 1::2], 'L3_hi_I', False, True)
                    qq = q3all[:, c0:c0 + 256]
                    mm(p3[:, 512:768], qq[:, 0::2], 'L3_q_R', True, False)
                    mm(p3[:, 512:768], qq[:, 1::2], 'L3_q_I', False, True)
                    y1z_s = midpool.tile([128, 256], F16, tag="y1z3")
                    nc.scalar.copy(y1z_s[:], p3[:, 0:256])
                    y1b_s = midpool.tile([128, 256], F16, tag="y1b3")
                    nc.vector.tensor_copy(out=y1b_s[:], in_=p3[:, 256:512])
                    y2b_s = midpool.tile([128, 256], F16, tag="y2b3")
                    nc.vector.tensor_copy(out=y2b_s[:], in_=p3[:, 512:768])

                    prow = ps3rpool.tile([128, 512], F32, tag="p3r")
                    for i in range(4):
                        img = base + i
                        zp = prow[:, i * 128:(i + 1) * 128]
                        zoff = (i % 2) * 64
                        zcol = (i // 2) * 128
                        mm(zp, y1z_s[zoff:zoff + 64, zcol:zcol + 128],
                           'M3_lo', True, False, poff=zoff)
                        if i < 3:
                            boff, sfx, bk = i * 32, '', 32
                        else:
                            boff, sfx, bk = 64, 'Z', 64
                        mm(zp, y1b_s[boff:boff + bk, 0:128],
                           'Be3_lo' + sfx, False, False, poff=boff)
                        mm(zp, y1b_s[boff:boff + bk, 128:256],
                           'Bo3_lo' + sfx, False, False, poff=boff)
                        mm(zp, y2b_s[boff:boff + bk, 0:128],
                           'Be3_hi' + sfx, False, False, poff=boff)
                        mm(zp, y2b_s[boff:boff + bk, 128:256],
                           'Bo3_hi' + sfx, False, True, poff=boff)
                        if i % 2 == 0:
                            nc.scalar.copy(z2s[img][:], zp)
                        else:
                            nc.vector.tensor_copy(out=z2s[img][:], in_=zp)

            # ===========================================================
            # Phase L2: z2 [128,128] + yh1 bands -> z1 [256,256], all imgs
            # ===========================================================
            with tc.tile_pool(name="ps2c", bufs=3, space="PSUM") as ps2cpool,\
                 tc.tile_pool(name="ps2r", bufs=2, space="PSUM") as ps2rpool:
                for img in range(IMGS_PER_CORE):
                    lh2 = lh2all[:, img * 128:(img + 1) * 128]
                    hl2 = hl2all[:, img * 128:(img + 1) * 128]
                    hh2 = hh2all[:, img * 128:(img + 1) * 128]
                    # phase A: p2 [128, 1024]: y1zT [0:256), b1 [512:1024)
                    p2a = ps2cpool.tile([128, 1024], F32, tag="p2")
                    mm(p2a[:, 0:256], z2s[img][:], 'M2_lo', True, True)
                    mm(p2a[0:64, 512:1024], lh2[:, 0::2], 'L2_hi_R',
                       True, False)
                    mm(p2a[0:64, 512:1024], lh2[:, 1::2], 'L2_hi_I',
                       False, True)
                    y1zT_s = midpool.tile([128, 256], F16, tag="y1zT2")
                    nc.scalar.copy(y1zT_s[:], p2a[:, 0:256])
                    # row2 = [b1; b2] K-stacked ([e | o] along columns)
                    row2 = midpool.tile([128, 512], F16, tag="row2")
                    nc.vector.tensor_copy(out=row2[0:64, :],
                                          in_=p2a[0:64, 512:1024])

                    # phase B (2nd buf): b2 [0:512)
                    p2b = ps2cpool.tile([128, 1024], F32, tag="p2")
                    mm(p2b[0:64, 0:512], hl2[:, 0::2], 'L2_lo_R',
                       True, False)
                    mm(p2b[0:64, 0:512], hl2[:, 1::2], 'L2_lo_I',
                       False, False)
                    mm(p2b[0:64, 0:512], hh2[:, 0::2], 'L2_hi_R',
                       False, False)
                    mm(p2b[0:64, 0:512], hh2[:, 1::2], 'L2_hi_I',
                       False, True)
                    b2_s = midpool.tile([64, 512], F16, tag="b2_2")
                    nc.vector.tensor_copy(out=b2_s[:], in_=p2b[0:64, 0:512])
                    nc.scalar.dma_start(out=row2[64:128, :], in_=b2_s[:])

                    # row stage -> z1 [256,256] as [128, 512]
                    p2r = ps2rpool.tile([128, 512], F32, tag="p2r")
                    for m in range(2):
                        zc = p2r[:, m * 256:(m + 1) * 256]
                        msl = slice(m * 128, (m + 1) * 128)
                        mm(zc, y1zT_s[:, msl], 'M2_lo', True, False)
                        mm(zc, row2[:, msl], 'R2_E', False, False)
                        mm(zc, row2[:, 256 + m * 128:256 + (m + 1) * 128],
                           'R2_O', False, True)
                        if m == 0:
                            nc.scalar.copy(z1s[img][:, 0:256], zc)
                        else:
                            nc.vector.tensor_copy(out=z1s[img][:, 256:512],
                                                  in_=zc)

            # ===========================================================
            # Phase L1: z1 [256,256] + yh0 bands -> out [256,256], all imgs
            # ===========================================================
            with tc.tile_pool(name="ps1c", bufs=3, space="PSUM") as ps1cpool,\
                 tc.tile_pool(name="ps1r", bufs=3, space="PSUM") as ps1rpool:
                for img in range(IMGS_PER_CORE):
                    o_t = {o: yh0all[:, img * 1536 + o * 256:
                                     img * 1536 + (o + 1) * 256]
                           for o in range(6)}
                    z1_s = z1s[img]
                    # phase A: y1 = band + lowpass, merged in w-polyphase
                    # layout [E(h 256) | O(h 256)]  (partitions = w')
                    p1a = ps1cpool.tile([128, 512], F32, tag="p1")
                    y1_p = p1a[:]
                    mm(y1_p, o_t[0][:, 0::2], 'L1hi_w1r', True, False)
                    mm(y1_p, o_t[5][:, 0::2], 'L1hi_w2r', False, False)
                    mm(y1_p, o_t[0][:, 1::2], 'L1hi_w1i', False, False)
                    mm(y1_p, o_t[5][:, 1::2], 'L1hi_w2i', False, False)
                    mm(p1a[:, 0:256], z1_s[:, 0:256:2], 'Alo_a',
                       False, False)
                    mm(p1a[:, 0:256], z1_s[:, 256:512:2], 'Alo_b',
                       False, True)
                    mm(p1a[:, 256:512], z1_s[:, 1:256:2], 'Alo_a',
                       False, False)
                    mm(p1a[:, 256:512], z1_s[:, 257:512:2], 'Alo_b',
                       False, True)
                    y1_s = midpool.tile([128, 512], F16, tag="y1m")
                    nc.vector.tensor_copy(out=y1_s[:], in_=y1_p)

                    # phase B: y2b e|o [0:512)
                    p1b = ps1cpool.tile([128, 512], F32, tag="p1")
                    y2b_p = p1b[:]
                    mm(y2b_p, o_t[2][:, 0::2], 'L1lo_w1r', True, False)
                    mm(y2b_p, o_t[3][:, 0::2], 'L1lo_w2r', False, False)
                    mm(y2b_p, o_t[2][:, 1::2], 'L1lo_w1i', False, False)
                    mm(y2b_p, o_t[3][:, 1::2], 'L1lo_w2i', False, False)
                    mm(y2b_p, o_t[1][:, 0::2], 'L1hi_w1r', False, False)
                    mm(y2b_p, o_t[4][:, 0::2], 'L1hi_w2r', False, False)
                    mm(y2b_p, o_t[1][:, 1::2], 'L1hi_w1i', False, False)
                    mm(y2b_p, o_t[4][:, 1::2], 'L1hi_w2i', False, True)
                    y2b1_s = midpool.tile([128, 512], F16, tag="y2b1")
                    nc.vector.tensor_copy(out=y2b1_s[:], in_=y2b_p)

                    # row stage -> out [256, 256] in two h-chunks; single
                    # store DMA per image ([a p] x <- p [a x])
                    p1r = ps1rpool.tile([128, 512], F32, tag="p1r")
                    ot = outpool.tile([128, 512], F16, tag="ot")
                    for m in range(2):
                        oc = p1r[:, m * 256:(m + 1) * 256]
                        msl = slice(m * 128, (m + 1) * 128)
                        osl = slice(256 + m * 128, 256 + (m + 1) * 128)
                        mm(oc, y1_s[:, msl], 'Be1_lo', True, False)
                        mm(oc, y1_s[:, osl], 'Bo1_lo', False, False)
                        mm(oc, y2b1_s[:, msl], 'Be1_hi', False, False)
                        mm(oc, y2b1_s[:, osl], 'Bo1_hi', False, True)
                        if m == 0:
                            nc.scalar.copy(ot[:, 0:256], oc)
                        else:
                            nc.vector.tensor_copy(out=ot[:, 256:512], in_=oc)
                    nc.gpsimd.dma_start(
                        out=out_d[img].rearrange("(a p) x -> p a x", a=2),
                        in_=ot.rearrange("p (a x) -> p a x", a=2))

    split_excess_waits(nc)
    return nc


# ---------------------------------------------------------------------------
# Entry point
# ---------------------------------------------------------------------------
_NC_CACHE = []
_LAST_RESULT = []  # last BassKernelResults (exec_time_ns when BASS_TRACE=1)


def _axon_reset():
    try:
        import ctypes
        lib = ctypes.CDLL('/opt/axon/libaxon_pjrt.so')
        lib.axon_reset.restype = ctypes.c_int64
        lib.axon_reset()
    except Exception:
        pass


def kernel(yl, yh0, yh1, yh2, g0o, g1o, g0a, g0b, g1a, g1b):
    yl = np.ascontiguousarray(np.asarray(yl, np.float16))
    yh0 = np.ascontiguousarray(np.asarray(yh0, np.float16))
    yh1 = np.ascontiguousarray(np.asarray(yh1, np.float16))
    yh2 = np.ascontiguousarray(np.asarray(yh2, np.float16))
    assert yl.shape == (8, 16, 64, 64)

    mats = build_matrices(g0o, g1o, g0a, g0b, g1a, g1b)
    blobA, blobB = pack_blobs(mats)
    if not _NC_CACHE:
        _NC_CACHE.append(build_nc())
    nc = _NC_CACHE[0]

    in_maps = []
    for core in range(N_CORES):
        m = {"yl": yl[core], "yh0": yh0[core],
             "yh1": yh1[core], "yh2": yh2[core],
             "matsA": blobA, "matsB": blobB}
        in_maps.append(m)

    try:
        res = run_bass_kernel_spmd(nc, in_maps, list(range(N_CORES)))
    except Exception as e:  # wedged exec unit: reset the axon device, retry
        if "UNAVAILABLE" not in str(e) and "unrecoverable" not in str(e):
            raise
        _axon_reset()
        res = run_bass_kernel_spmd(nc, in_maps, list(range(N_CORES)))
    _LAST_RESULT.clear()
    _LAST_RESULT.append(res)
    out = np.stack([res.results[i]["out"] for i in range(N_CORES)], axis=0)
    return np.ascontiguousarray(out.astype(np.float32))


# revision 22
# speedup vs baseline: 1.1570x; 1.1570x over previous
"""DTCWT 3-level inverse on 8 Trainium2 NeuronCores.

Every filtering stage is a banded matmul on the tensor engine in fp16
(PSUM accumulates fp32; ~7e-4 total rel err vs the 2e-2 gate).

All stages use "data as lhsT" mode: matmul(out, lhsT=data[K=h, M=w],
rhs=mat[K=h, N=h_out]) contracts over the partition dim of the data and
yields the filtered image TRANSPOSED ([w, h_out]); column and row stages
then alternate orientation naturally with zero explicit transposes.

The c2q band construction is folded into the matrices; at L1 the lowpass
path is additionally merged into the band polyphase layout ([E|O] w-planes)
so the final row stage is 4 accumulation passes instead of 6.

Schedule: phase-major (L3 x16, L2 x16, L1 x16) with double/triple-buffered
PSUM pools so neighbouring images' matmuls hide each other's copy latency.
DMA queue slots cost ~600ns regardless of size, so all loads are batched
into a handful of giant multi-dim DMAs (2 matrix blobs, 9 input sweeps,
1 store per image).

Sharding: pure data parallel over batch N (8 cores x 16 channels each).
"""
import sys

for _p in ('/opt/trn_rl_repo',):
    if _p not in sys.path:
        sys.path.append(_p)

import numpy as np
import concourse.bass as bass
import concourse.mybir as mybir
from concourse.tile import TileContext
from concourse.bass_utils import run_bass_kernel_spmd

SQRT_HALF = 0.7071067811865476
N_CORES = 8
IMGS_PER_CORE = 16
F32 = mybir.dt.float32
F16 = mybir.dt.float16


# ---------------------------------------------------------------------------
# Host-side matrix construction (numpy, float64)
# ---------------------------------------------------------------------------
def _conv_rows_valid(x, h):
    hr = h[::-1]
    taps = h.shape[0]
    n = x.shape[-2] - taps + 1
    out = hr[0] * x[..., 0:n, :]
    for k in range(1, taps):
        out = out + hr[k] * x[..., k:k + n, :]
    return out


def _pad_rows_symmetric(x, m):
    pad = [(0, 0)] * (x.ndim - 2) + [(m, m), (0, 0)]
    return np.pad(x, pad, mode='symmetric')


def _colfilter(x, h):
    return _conv_rows_valid(_pad_rows_symmetric(x, h.shape[0] // 2), h)


def _colifilt(x, ha, hb, highpass):
    m = ha.shape[0]
    m2 = m // 2
    r = x.shape[-2]
    xp = _pad_rows_symmetric(x, m2)
    xe = xp[..., 1:r + m - 2:2, :]
    xo = xp[..., 2:r + m - 1:2, :]
    xa, xb = (xe, xo) if highpass else (xo, xe)
    hao, hae = ha[0::2], ha[1::2]
    hbo, hbe = hb[0::2], hb[1::2]
    y0 = _conv_rows_valid(xb, hao)
    y1 = _conv_rows_valid(xa, hbo)
    y2 = _conv_rows_valid(xb, hae)
    y3 = _conv_rows_valid(xa, hbe)
    y = np.stack([y0, y1, y2, y3], axis=-2)
    return y.reshape(y.shape[:-3] + (2 * r, y.shape[-1]))


def _op_matrix(op, n):
    """M[h_in, h_out] with out[h_out, w] = sum_h M[h, h_out] x[h, w]."""
    return np.ascontiguousarray(op(np.eye(n, dtype=np.float64)).T)


def build_matrices(g0o, g1o, g0a, g0b, g1a, g1b):
    """All device matrices as {name: fp16 ndarray}."""
    g0o = np.asarray(g0o, np.float64)
    g1o = np.asarray(g1o, np.float64)
    g0a = np.asarray(g0a, np.float64)
    g0b = np.asarray(g0b, np.float64)
    g1a = np.asarray(g1a, np.float64)
    g1b = np.asarray(g1b, np.float64)
    s = SQRT_HALF
    hs, vs = np.hstack, np.vstack
    out = {}

    def upsample_level(R, tag):
        Mlo = _op_matrix(lambda x: _colifilt(x, g0b, g0a, False), R)  # [R, 2R]
        Mhi = _op_matrix(lambda x: _colifilt(x, g1b, g1a, True), R)
        Me_h, Mo_h = s * Mhi[0::2], s * Mhi[1::2]                     # [R/2, 2R]
        Me_l, Mo_l = s * Mlo[0::2], s * Mlo[1::2]
        out[f'M{tag}_lo'] = Mlo
        # pair-stacked [w1; w2] col rhs, e|o column-concatenated
        #   e: w1r*Me + w2r*Me + w1i*Mo - w2i*Mo
        #   o: -w1r*Mo + w2r*Mo + w1i*Me + w2i*Me
        out[f'L{tag}_hi_R'] = hs([vs([Me_h, Me_h]), vs([-Mo_h, Mo_h])])
        out[f'L{tag}_hi_I'] = hs([vs([Mo_h, -Mo_h]), vs([Me_h, Me_h])])
        out[f'L{tag}_lo_R'] = hs([vs([Me_l, Me_l]), vs([-Mo_l, Mo_l])])
        out[f'L{tag}_lo_I'] = hs([vs([Mo_l, -Mo_l]), vs([Me_l, Me_l])])
        # row stage (polyphase-column recombination)
        out[f'Be{tag}_lo'], out[f'Bo{tag}_lo'] = Mlo[0::2], Mlo[1::2]
        out[f'Be{tag}_hi'], out[f'Bo{tag}_hi'] = Mhi[0::2], Mhi[1::2]

    upsample_level(64, '3')
    upsample_level(128, '2')
    # L3 quad stacks: [hl pair (lo mats); hh pair (hi mats)], K=128
    out['L3_q_R'] = vs([out['L3_lo_R'], out['L3_hi_R']])
    out['L3_q_I'] = vs([out['L3_lo_I'], out['L3_hi_I']])
    del out['L3_lo_R'], out['L3_lo_I']  # only used inside the quad at L3
    # K-stacked row-stage rhs at L2 (lhsT pieces partition-stacked)
    out['R2_E'] = vs([out['Be2_lo'], out['Be2_hi']])
    out['R2_O'] = vs([out['Bo2_lo'], out['Bo2_hi']])
    for k in ('Be2_lo', 'Bo2_lo', 'Be2_hi', 'Bo2_hi'):
        del out[k]

    # L1 (colfilter, size-preserving, n=256)
    A_lo = _op_matrix(lambda x: _colfilter(x, g0o), 256)              # [256, 256]
    A_hi = _op_matrix(lambda x: _colfilter(x, g1o), 256)
    out['Alo_a'], out['Alo_b'] = A_lo[0:128], A_lo[128:256]
    for x, A in (('hi', A_hi), ('lo', A_lo)):
        Me, Mo = s * A[0::2], s * A[1::2]                             # [128, 256]
        out[f'L1{x}_w1r'] = hs([Me, -Mo])
        out[f'L1{x}_w2r'] = hs([Me, Mo])
        out[f'L1{x}_w1i'] = hs([Mo, Me])
        out[f'L1{x}_w2i'] = hs([-Mo, Me])
    out['Be1_lo'], out['Bo1_lo'] = A_lo[0::2], A_lo[1::2]
    out['Be1_hi'], out['Bo1_hi'] = A_hi[0::2], A_hi[1::2]
    # zero-top variants: lhsT base partitions are limited to {0,32,64}, so
    # the 4th 32-row band slot (base 96) runs as K=64 at base 64 with the
    # top half of the matrix zeroed.
    z32 = np.zeros((32, 128))
    for nm in ('Be3_lo', 'Bo3_lo', 'Be3_hi', 'Bo3_hi'):
        out[nm + 'Z'] = np.vstack([z32, out[nm]])
    return {k: np.ascontiguousarray(v, np.float16) for k, v in out.items()}


MAT_SHAPES = {
    'M3_lo': (64, 128),
    'L3_hi_R': (64, 256), 'L3_hi_I': (64, 256),
    'L3_q_R': (128, 256), 'L3_q_I': (128, 256),
    'Be3_lo': (32, 128), 'Bo3_lo': (32, 128),
    'Be3_hi': (32, 128), 'Bo3_hi': (32, 128),
    'Be3_loZ': (64, 128), 'Bo3_loZ': (64, 128),
    'Be3_hiZ': (64, 128), 'Bo3_hiZ': (64, 128),
    'M2_lo': (128, 256),
    'L2_hi_R': (128, 512), 'L2_hi_I': (128, 512),
    'L2_lo_R': (128, 512), 'L2_lo_I': (128, 512),
    'R2_E': (128, 256), 'R2_O': (128, 256),
    'Alo_a': (128, 256), 'Alo_b': (128, 256),
    'L1hi_w1r': (128, 512), 'L1hi_w2r': (128, 512),
    'L1hi_w1i': (128, 512), 'L1hi_w2i': (128, 512),
    'L1lo_w1r': (128, 512), 'L1lo_w2r': (128, 512),
    'L1lo_w1i': (128, 512), 'L1lo_w2i': (128, 512),
    'Be1_lo': (128, 256), 'Bo1_lo': (128, 256),
    'Be1_hi': (128, 256), 'Bo1_hi': (128, 256),
}

BLOB_A = ['M3_lo', 'L3_hi_R', 'L3_hi_I', 'L3_q_R', 'L3_q_I',
          'Be3_lo', 'Bo3_lo', 'Be3_hi', 'Bo3_hi',
          'Be3_loZ', 'Bo3_loZ', 'Be3_hiZ', 'Bo3_hiZ',
          'M2_lo', 'L2_hi_R', 'L2_hi_I', 'L2_lo_R', 'L2_lo_I',
          'R2_E', 'R2_O']
BLOB_B = ['Alo_a', 'Alo_b',
          'L1hi_w1r', 'L1hi_w2r', 'L1hi_w1i', 'L1hi_w2i',
          'L1lo_w1r', 'L1lo_w2r', 'L1lo_w1i', 'L1lo_w2i',
          'Be1_lo', 'Bo1_lo', 'Be1_hi', 'Bo1_hi']
BLOB_A_COLS = sum(MAT_SHAPES[n][1] for n in BLOB_A)
BLOB_B_COLS = sum(MAT_SHAPES[n][1] for n in BLOB_B)


def pack_blobs(mats):
    def pack(names, cols):
        blob = np.zeros((128, cols), np.float16)
        c = 0
        for n in names:
            K, N = MAT_SHAPES[n]
            for r in range(128 // K):  # replicate K<128 mats across parts
                blob[r * K:(r + 1) * K, c:c + N] = mats[n]
            c += N
        return blob
    return pack(BLOB_A, BLOB_A_COLS), pack(BLOB_B, BLOB_B_COLS)


# ---------------------------------------------------------------------------
# Bass kernel
# ---------------------------------------------------------------------------
def split_excess_waits(nc, max_waits=1):
    """walrus CTRL codegen allows only one sem wait per instruction; move
    excess waits onto NoOps inserted just before the offending instruction."""
    ctr = 0
    for fn in nc.m.functions:
        for bb in fn.blocks:
            new_list = []
            for inst in bb.instructions:
                si = inst.sync_info
                if si is not None and si.on_wait and len(si.on_wait) > max_waits:
                    waits = list(si.on_wait)
                    keep, extra = waits[:max_waits], waits[max_waits:]
                    for i in range(0, len(extra), max_waits):
                        nop = mybir.InstNoOp(
                            name=f"wait_split_{ctr}", ins=[], outs=[])
                        ctr += 1
                        nop.engine = inst.engine
                        nop.sync_info = mybir.SyncInfo(
                            on_wait=extra[i:i + max_waits], on_update=[])
                        nc.register_instruction(nop)
                        new_list.append(nop)
                    inst.sync_info = mybir.SyncInfo(
                        on_wait=keep,
                        on_update=list(si.on_update) if si.on_update else [])
                new_list.append(inst)
            bb.instructions[:] = new_list
    return ctr


def build_nc():
    nc = bass.Bass()
    yl_d = nc.dram_tensor("yl", [IMGS_PER_CORE, 64, 64], F16,
                          kind="ExternalInput")
    yh2_d = nc.dram_tensor("yh2", [IMGS_PER_CORE, 6, 32, 32, 2], F16,
                           kind="ExternalInput")
    yh1_d = nc.dram_tensor("yh1", [IMGS_PER_CORE, 6, 64, 64, 2], F16,
                           kind="ExternalInput")
    yh0_d = nc.dram_tensor("yh0", [IMGS_PER_CORE, 6, 128, 128, 2], F16,
                           kind="ExternalInput")
    out_d = nc.dram_tensor("out", [IMGS_PER_CORE, 256, 256], F16,
                           kind="ExternalOutput")
    matsA_d = nc.dram_tensor("matsA", [128, BLOB_A_COLS], F16,
                             kind="ExternalInput")
    matsB_d = nc.dram_tensor("matsB", [128, BLOB_B_COLS], F16,
                             kind="ExternalInput")

    with TileContext(nc) as tc:
        with tc.tile_pool(name="mats", bufs=1) as matpool, \
             tc.tile_pool(name="ins", bufs=1) as inpool, \
             tc.tile_pool(name="zs", bufs=1) as zpool, \
             tc.tile_pool(name="mid", bufs=3) as midpool, \
             tc.tile_pool(name="outp", bufs=4) as outpool:

            # --- matrix blobs: L3 head first so img0 can start early ---
            L3_HEAD = 2176  # cols of the 9 L3 matrices at the blobA front
            blobA_t = matpool.tile([128, BLOB_A_COLS], F16, tag="blobA")
            nc.scalar.dma_start(out=blobA_t[:, 0:L3_HEAD],
                                in_=matsA_d[:, 0:L3_HEAD])
            nc.scalar.dma_start(out=blobA_t[:, L3_HEAD:],
                                in_=matsA_d[:, L3_HEAD:])
            blobB_t = matpool.tile([128, BLOB_B_COLS], F16, tag="blobB")
            nc.sync.dma_start(out=blobB_t[:], in_=matsB_d[:])
            mats = {}
            mat_loc = {}
            cA = cB = 0
            for n in BLOB_A:
                K, N = MAT_SHAPES[n]
                mats[n] = blobA_t[0:K, cA:cA + N]
                mat_loc[n] = (blobA_t, cA)
                cA += N
            for n in BLOB_B:
                K, N = MAT_SHAPES[n]
                mats[n] = blobB_t[0:K, cB:cB + N]
                mat_loc[n] = (blobB_t, cB)
                cB += N

            def mat_at(name, poff):
                blob, c = mat_loc[name]
                K, N = MAT_SHAPES[name]
                return blob[poff:poff + K, c:c + N]

            # --- batched input sweeps ---
            # z3all[h, (i w)] <- yl[i, h, w]
            z3all = inpool.tile([64, 16 * 64], F16, tag="z3all")
            nc.gpsimd.dma_start(
                out=z3all.rearrange("h (i x) -> h i x", i=16),
                in_=yl_d.rearrange("i h x -> h i x"))
            # per-orientation all-image sweeps (DMA APs max 3 dims)
            def band_sweep(tile_ap, p0, p1, src5, i=16):
                nc.gpsimd.dma_start(
                    out=tile_ap[p0:p1, :].rearrange("h (i x) -> h i x", i=i),
                    in_=src5.rearrange("i h w r -> h i (w r)"))
            lh3all = inpool.tile([64, 16 * 64], F16, tag="lh3all")
            band_sweep(lh3all, 0, 32, yh2_d[:, 0])
            band_sweep(lh3all, 32, 64, yh2_d[:, 5])
            # q3all: parts 0:64 = orient pair (2,3), 64:128 = (1,4)
            q3all = inpool.tile([128, 16 * 64], F16, tag="q3all")
            band_sweep(q3all, 0, 32, yh2_d[:, 2])
            band_sweep(q3all, 32, 64, yh2_d[:, 3])
            band_sweep(q3all, 64, 96, yh2_d[:, 1])
            band_sweep(q3all, 96, 128, yh2_d[:, 4])
            # yh1 band pair tiles, one DMA per orientation
            lh2all = inpool.tile([128, 16 * 128], F16, tag="lh2all")
            band_sweep(lh2all, 0, 64, yh1_d[:, 0])
            band_sweep(lh2all, 64, 128, yh1_d[:, 5])
            hl2all = inpool.tile([128, 16 * 128], F16, tag="hl2all")
            band_sweep(hl2all, 0, 64, yh1_d[:, 2])
            band_sweep(hl2all, 64, 128, yh1_d[:, 3])
            hh2all = inpool.tile([128, 16 * 128], F16, tag="hh2all")
            band_sweep(hh2all, 0, 64, yh1_d[:, 1])
            band_sweep(hh2all, 64, 128, yh1_d[:, 4])
            # yh0: 4 groups of 4 imgs on the sync queue
            yh0all = inpool.tile([128, 16 * 1536], F16, tag="yh0all")
            for g in range(4):
                nc.sync.dma_start(
                    out=yh0all[:, g * 6144:(g + 1) * 6144].rearrange(
                        "h (g x) -> h g x", g=24),
                    in_=yh0_d[4 * g:4 * g + 4].rearrange(
                        "i o h w r -> h (i o) (w r)"))

            z2s = {img: zpool.tile([128, 128], F16, tag=f"z2_{img}",
                                   name=f"z2_{img}")
                   for img in range(IMGS_PER_CORE)}
            z1s = {img: zpool.tile([128, 512], F16, tag=f"z1_{img}",
                                   name=f"z1_{img}")
                   for img in range(IMGS_PER_CORE)}

            def mm(out_ap, lhsT, rhs_name, start, stop, poff=0):
                rhs = mats[rhs_name] if poff == 0 else mat_at(rhs_name, poff)
                nc.tensor.matmul(out_ap, lhsT, rhs, start=start, stop=stop)

            # ===========================================================
            # Phase L3: quad-packed (M=128 across 4 imgs) col stages,
            # per-img row stages via lhsT partition offsets
            # ===========================================================
            with tc.tile_pool(name="ps3c", bufs=2, space="PSUM") as ps3cpool,\
                 tc.tile_pool(name="ps3r", bufs=2, space="PSUM") as ps3rpool:
                for g in range(4):
                    base = 4 * g
                    c0 = base * 64
                    p3 = ps3cpool.tile([128, 1024], F32, tag="p3")
                    mm(p3[:, 0:128], z3all[:, c0:c0 + 128], 'M3_lo',
                       True, True)
                    mm(p3[:, 128:256], z3all[:, c0 + 128:c0 + 256], 'M3_lo',
                       True, True)
                    lq = lh3all[:, c0:c0 + 256]
                    mm(p3[:, 256:512], lq[:, 0::2], 'L3_hi_R', True, False)
                    mm(p3[:, 256:512], lq[:, 1::2], 'L3_hi_I', False, True)
                    qq = q3all[:, c0:c0 + 256]
                    mm(p3[:, 512:768], qq[:, 0::2], 'L3_q_R', True, False)
                    mm(p3[:, 512:768], qq[:, 1::2], 'L3_q_I', False, True)
                    y1z_s = midpool.tile([128, 256], F16, tag="y1z3")
                    nc.scalar.copy(y1z_s[:], p3[:, 0:256])
                    y1b_s = midpool.tile([128, 256], F16, tag="y1b3")
                    nc.vector.tensor_copy(out=y1b_s[:], in_=p3[:, 256:512])
                    y2b_s = midpool.tile([128, 256], F16, tag="y2b3")
                    nc.vector.tensor_copy(out=y2b_s[:], in_=p3[:, 512:768])

                    prow = ps3rpool.tile([128, 512], F32, tag="p3r")
                    for i in range(4):
                        img = base + i
                        zp = prow[:, i * 128:(i + 1) * 128]
                        zoff = (i % 2) * 64
                        zcol = (i // 2) * 128
                        mm(zp, y1z_s[zoff:zoff + 64, zcol:zcol + 128],
                           'M3_lo', True, False, poff=zoff)
                        if i < 3:
                            boff, sfx, bk = i * 32, '', 32
                        else:
                            boff, sfx, bk = 64, 'Z', 64
                        mm(zp, y1b_s[boff:boff + bk, 0:128],
                           'Be3_lo' + sfx, False, False, poff=boff)
                        mm(zp, y1b_s[boff:boff + bk, 128:256],
                           'Bo3_lo' + sfx, False, False, poff=boff)
                        mm(zp, y2b_s[boff:boff + bk, 0:128],
                           'Be3_hi' + sfx, False, False, poff=boff)
                        mm(zp, y2b_s[boff:boff + bk, 128:256],
                           'Bo3_hi' + sfx, False, True, poff=boff)
                        if i % 2 == 0:
                            nc.scalar.copy(z2s[img][:], zp)
                        else:
                            nc.vector.tensor_copy(out=z2s[img][:], in_=zp)

            # ===========================================================
            # Phase L2: z2 [128,128] + yh1 bands -> z1 [256,256], all imgs
            # ===========================================================
            with tc.tile_pool(name="ps2c", bufs=2, space="PSUM") as ps2cpool,\
                 tc.tile_pool(name="ps2r", bufs=2, space="PSUM") as ps2rpool:
                for img in range(IMGS_PER_CORE):
                    lh2 = lh2all[:, img * 128:(img + 1) * 128]
                    hl2 = hl2all[:, img * 128:(img + 1) * 128]
                    hh2 = hh2all[:, img * 128:(img + 1) * 128]
                    # phase A: p2 [128, 1024]: y1zT [0:256), b1 [512:1024)
                    p2a = ps2cpool.tile([128, 1024], F32, tag="p2")
                    mm(p2a[:, 0:256], z2s[img][:], 'M2_lo', True, True)
                    mm(p2a[0:64, 512:1024], lh2[:, 0::2], 'L2_hi_R',
                       True, False)
                    mm(p2a[0:64, 512:1024], lh2[:, 1::2], 'L2_hi_I',
                       False, True)
                    y1zT_s = midpool.tile([128, 256], F16, tag="y1zT2")
                    nc.scalar.copy(y1zT_s[:], p2a[:, 0:256])
                    # row2 = [b1; b2] K-stacked ([e | o] along columns)
                    row2 = midpool.tile([128, 512], F16, tag="row2")
                    nc.vector.tensor_copy(out=row2[0:64, :],
                                          in_=p2a[0:64, 512:1024])

                    # phase B (2nd buf): b2 [0:512)
                    p2b = ps2cpool.tile([128, 1024], F32, tag="p2")
                    mm(p2b[0:64, 0:512], hl2[:, 0::2], 'L2_lo_R',
                       True, False)
                    mm(p2b[0:64, 0:512], hl2[:, 1::2], 'L2_lo_I',
                       False, False)
                    mm(p2b[0:64, 0:512], hh2[:, 0::2], 'L2_hi_R',
                       False, False)
                    mm(p2b[0:64, 0:512], hh2[:, 1::2], 'L2_hi_I',
                       False, True)
                    b2_s = midpool.tile([64, 512], F16, tag="b2_2")
                    nc.vector.tensor_copy(out=b2_s[:], in_=p2b[0:64, 0:512])
                    nc.scalar.dma_start(out=row2[64:128, :], in_=b2_s[:])

                    # row stage -> z1 [256,256] as [128, 512]
                    p2r = ps2rpool.tile([128, 512], F32, tag="p2r")
                    for m in range(2):
                        zc = p2r[:, m * 256:(m + 1) * 256]
                        msl = slice(m * 128, (m + 1) * 128)
                        mm(zc, y1zT_s[:, msl], 'M2_lo', True, False)
                        mm(zc, row2[:, msl], 'R2_E', False, False)
                        mm(zc, row2[:, 256 + m * 128:256 + (m + 1) * 128],
                           'R2_O', False, True)
                        if m == 0:
                            nc.scalar.copy(z1s[img][:, 0:256], zc)
                        else:
                            nc.vector.tensor_copy(out=z1s[img][:, 256:512],
                                                  in_=zc)

            # ===========================================================
            # Phase L1: z1 [256,256] + yh0 bands -> out [256,256], all imgs
            # ===========================================================
            with tc.tile_pool(name="ps1c", bufs=3, space="PSUM") as ps1cpool,\
                 tc.tile_pool(name="ps1r", bufs=2, space="PSUM") as ps1rpool:
                for img in range(IMGS_PER_CORE):
                    o_t = {o: yh0all[:, img * 1536 + o * 256:
                                     img * 1536 + (o + 1) * 256]
                           for o in range(6)}
                    z1_s = z1s[img]
                    # phase A: y1 = band + lowpass, merged in w-polyphase
                    # layout [E(h 256) | O(h 256)]  (partitions = w')
                    p1a = ps1cpool.tile([128, 512], F32, tag="p1")
                    y1_p = p1a[:]
                    mm(y1_p, o_t[0][:, 0::2], 'L1hi_w1r', True, False)
                    mm(y1_p, o_t[5][:, 0::2], 'L1hi_w2r', False, False)
                    mm(y1_p, o_t[0][:, 1::2], 'L1hi_w1i', False, False)
                    mm(y1_p, o_t[5][:, 1::2], 'L1hi_w2i', False, False)
                    mm(p1a[:, 0:256], z1_s[:, 0:256:2], 'Alo_a',
                       False, False)
                    mm(p1a[:, 0:256], z1_s[:, 256:512:2], 'Alo_b',
                       False, True)
                    mm(p1a[:, 256:512], z1_s[:, 1:256:2], 'Alo_a',
                       False, False)
                    mm(p1a[:, 256:512], z1_s[:, 257:512:2], 'Alo_b',
                       False, True)
                    y1_s = midpool.tile([128, 512], F16, tag="y1m")
                    nc.vector.tensor_copy(out=y1_s[:], in_=y1_p)

                    # phase B: y2b e|o [0:512)
                    p1b = ps1cpool.tile([128, 512], F32, tag="p1")
                    y2b_p = p1b[:]
                    mm(y2b_p, o_t[2][:, 0::2], 'L1lo_w1r', True, False)
                    mm(y2b_p, o_t[3][:, 0::2], 'L1lo_w2r', False, False)
                    mm(y2b_p, o_t[2][:, 1::2], 'L1lo_w1i', False, False)
                    mm(y2b_p, o_t[3][:, 1::2], 'L1lo_w2i', False, False)
                    mm(y2b_p, o_t[1][:, 0::2], 'L1hi_w1r', False, False)
                    mm(y2b_p, o_t[4][:, 0::2], 'L1hi_w2r', False, False)
                    mm(y2b_p, o_t[1][:, 1::2], 'L1hi_w1i', False, False)
                    mm(y2b_p, o_t[4][:, 1::2], 'L1hi_w2i', False, True)
                    y2b1_s = midpool.tile([128, 512], F16, tag="y2b1")
                    nc.vector.tensor_copy(out=y2b1_s[:], in_=y2b_p)

                    # row stage -> out [256, 256] in two h-chunks; single
                    # store DMA per image ([a p] x <- p [a x])
                    p1r = ps1rpool.tile([128, 512], F32, tag="p1r")
                    ot = outpool.tile([128, 512], F16, tag="ot")
                    for m in range(2):
                        oc = p1r[:, m * 256:(m + 1) * 256]
                        msl = slice(m * 128, (m + 1) * 128)
                        osl = slice(256 + m * 128, 256 + (m + 1) * 128)
                        mm(oc, y1_s[:, msl], 'Be1_lo', True, False)
                        mm(oc, y1_s[:, osl], 'Bo1_lo', False, False)
                        mm(oc, y2b1_s[:, msl], 'Be1_hi', False, False)
                        mm(oc, y2b1_s[:, osl], 'Bo1_hi', False, True)
                        if m == 0:
                            nc.scalar.copy(ot[:, 0:256], oc)
                        else:
                            nc.vector.tensor_copy(out=ot[:, 256:512], in_=oc)
                    nc.gpsimd.dma_start(
                        out=out_d[img].rearrange("(a p) x -> p a x", a=2),
                        in_=ot.rearrange("p (a x) -> p a x", a=2))

    split_excess_waits(nc)
    return nc


# ---------------------------------------------------------------------------
# Entry point
# ---------------------------------------------------------------------------
_NC_CACHE = []
_LAST_RESULT = []  # last BassKernelResults (exec_time_ns when BASS_TRACE=1)


def _axon_reset():
    try:
        import ctypes
        lib = ctypes.CDLL('/opt/axon/libaxon_pjrt.so')
        lib.axon_reset.restype = ctypes.c_int64
        lib.axon_reset()
    except Exception:
        pass


def kernel(yl, yh0, yh1, yh2, g0o, g1o, g0a, g0b, g1a, g1b):
    yl = np.ascontiguousarray(np.asarray(yl, np.float16))
    yh0 = np.ascontiguousarray(np.asarray(yh0, np.float16))
    yh1 = np.ascontiguousarray(np.asarray(yh1, np.float16))
    yh2 = np.ascontiguousarray(np.asarray(yh2, np.float16))
    assert yl.shape == (8, 16, 64, 64)

    mats = build_matrices(g0o, g1o, g0a, g0b, g1a, g1b)
    blobA, blobB = pack_blobs(mats)
    if not _NC_CACHE:
        _NC_CACHE.append(build_nc())
    nc = _NC_CACHE[0]

    in_maps = []
    for core in range(N_CORES):
        m = {"yl": yl[core], "yh0": yh0[core],
             "yh1": yh1[core], "yh2": yh2[core],
             "matsA": blobA, "matsB": blobB}
        in_maps.append(m)

    try:
        res = run_bass_kernel_spmd(nc, in_maps, list(range(N_CORES)))
    except Exception as e:  # wedged exec unit: reset the axon device, retry
        if "UNAVAILABLE" not in str(e) and "unrecoverable" not in str(e):
            raise
        _axon_reset()
        res = run_bass_kernel_spmd(nc, in_maps, list(range(N_CORES)))
    _LAST_RESULT.clear()
    _LAST_RESULT.append(res)
    out = np.stack([res.results[i]["out"] for i in range(N_CORES)], axis=0)
    return np.ascontiguousarray(out.astype(np.float32))


# revision 23
# speedup vs baseline: 1.2238x; 1.0578x over previous
"""DTCWT 3-level inverse on 8 Trainium2 NeuronCores.

Every filtering stage is a banded matmul on the tensor engine in fp16
(PSUM accumulates fp32; ~7e-4 total rel err vs the 2e-2 gate).

All stages use "data as lhsT" mode: matmul(out, lhsT=data[K=h, M=w],
rhs=mat[K=h, N=h_out]) contracts over the partition dim of the data and
yields the filtered image TRANSPOSED ([w, h_out]); column and row stages
then alternate orientation naturally with zero explicit transposes.

The c2q band construction is folded into the matrices; at L1 the lowpass
path is additionally merged into the band polyphase layout ([E|O] w-planes)
so the final row stage is 4 accumulation passes instead of 6.

Schedule: phase-major (L3 x16, L2 x16, L1 x16) with double/triple-buffered
PSUM pools so neighbouring images' matmuls hide each other's copy latency.
DMA queue slots cost ~600ns regardless of size, so all loads are batched
into a handful of giant multi-dim DMAs (2 matrix blobs, 9 input sweeps,
1 store per image).

Sharding: pure data parallel over batch N (8 cores x 16 channels each).
"""
import sys

for _p in ('/opt/trn_rl_repo',):
    if _p not in sys.path:
        sys.path.append(_p)

import numpy as np
import concourse.bass as bass
import concourse.mybir as mybir
from concourse.tile import TileContext
from concourse.bass_utils import run_bass_kernel_spmd

SQRT_HALF = 0.7071067811865476
N_CORES = 8
IMGS_PER_CORE = 16
F32 = mybir.dt.float32
F16 = mybir.dt.float16


# ---------------------------------------------------------------------------
# Host-side matrix construction (numpy, float64)
# ---------------------------------------------------------------------------
def _conv_rows_valid(x, h):
    hr = h[::-1]
    taps = h.shape[0]
    n = x.shape[-2] - taps + 1
    out = hr[0] * x[..., 0:n, :]
    for k in range(1, taps):
        out = out + hr[k] * x[..., k:k + n, :]
    return out


def _pad_rows_symmetric(x, m):
    pad = [(0, 0)] * (x.ndim - 2) + [(m, m), (0, 0)]
    return np.pad(x, pad, mode='symmetric')


def _colfilter(x, h):
    return _conv_rows_valid(_pad_rows_symmetric(x, h.shape[0] // 2), h)


def _colifilt(x, ha, hb, highpass):
    m = ha.shape[0]
    m2 = m // 2
    r = x.shape[-2]
    xp = _pad_rows_symmetric(x, m2)
    xe = xp[..., 1:r + m - 2:2, :]
    xo = xp[..., 2:r + m - 1:2, :]
    xa, xb = (xe, xo) if highpass else (xo, xe)
    hao, hae = ha[0::2], ha[1::2]
    hbo, hbe = hb[0::2], hb[1::2]
    y0 = _conv_rows_valid(xb, hao)
    y1 = _conv_rows_valid(xa, hbo)
    y2 = _conv_rows_valid(xb, hae)
    y3 = _conv_rows_valid(xa, hbe)
    y = np.stack([y0, y1, y2, y3], axis=-2)
    return y.reshape(y.shape[:-3] + (2 * r, y.shape[-1]))


def _op_matrix(op, n):
    """M[h_in, h_out] with out[h_out, w] = sum_h M[h, h_out] x[h, w]."""
    return np.ascontiguousarray(op(np.eye(n, dtype=np.float64)).T)


def build_matrices(g0o, g1o, g0a, g0b, g1a, g1b):
    """All device matrices as {name: fp16 ndarray}."""
    g0o = np.asarray(g0o, np.float64)
    g1o = np.asarray(g1o, np.float64)
    g0a = np.asarray(g0a, np.float64)
    g0b = np.asarray(g0b, np.float64)
    g1a = np.asarray(g1a, np.float64)
    g1b = np.asarray(g1b, np.float64)
    s = SQRT_HALF
    hs, vs = np.hstack, np.vstack
    out = {}

    def upsample_level(R, tag):
        Mlo = _op_matrix(lambda x: _colifilt(x, g0b, g0a, False), R)  # [R, 2R]
        Mhi = _op_matrix(lambda x: _colifilt(x, g1b, g1a, True), R)
        Me_h, Mo_h = s * Mhi[0::2], s * Mhi[1::2]                     # [R/2, 2R]
        Me_l, Mo_l = s * Mlo[0::2], s * Mlo[1::2]
        out[f'M{tag}_lo'] = Mlo
        # pair-stacked [w1; w2] col rhs, e|o column-concatenated
        #   e: w1r*Me + w2r*Me + w1i*Mo - w2i*Mo
        #   o: -w1r*Mo + w2r*Mo + w1i*Me + w2i*Me
        out[f'L{tag}_hi_R'] = hs([vs([Me_h, Me_h]), vs([-Mo_h, Mo_h])])
        out[f'L{tag}_hi_I'] = hs([vs([Mo_h, -Mo_h]), vs([Me_h, Me_h])])
        out[f'L{tag}_lo_R'] = hs([vs([Me_l, Me_l]), vs([-Mo_l, Mo_l])])
        out[f'L{tag}_lo_I'] = hs([vs([Mo_l, -Mo_l]), vs([Me_l, Me_l])])
        # row stage (polyphase-column recombination)
        out[f'Be{tag}_lo'], out[f'Bo{tag}_lo'] = Mlo[0::2], Mlo[1::2]
        out[f'Be{tag}_hi'], out[f'Bo{tag}_hi'] = Mhi[0::2], Mhi[1::2]

    upsample_level(64, '3')
    upsample_level(128, '2')
    # L3 quad stacks: [hl pair (lo mats); hh pair (hi mats)], K=128
    out['L3_q_R'] = vs([out['L3_lo_R'], out['L3_hi_R']])
    out['L3_q_I'] = vs([out['L3_lo_I'], out['L3_hi_I']])
    del out['L3_lo_R'], out['L3_lo_I']  # only used inside the quad at L3

    # L1 (colfilter, size-preserving, n=256)
    A_lo = _op_matrix(lambda x: _colfilter(x, g0o), 256)              # [256, 256]
    A_hi = _op_matrix(lambda x: _colfilter(x, g1o), 256)
    out['Alo_a'], out['Alo_b'] = A_lo[0:128], A_lo[128:256]
    for x, A in (('hi', A_hi), ('lo', A_lo)):
        Me, Mo = s * A[0::2], s * A[1::2]                             # [128, 256]
        out[f'L1{x}_w1r'] = hs([Me, -Mo])
        out[f'L1{x}_w2r'] = hs([Me, Mo])
        out[f'L1{x}_w1i'] = hs([Mo, Me])
        out[f'L1{x}_w2i'] = hs([-Mo, Me])
    out['Be1_lo'], out['Bo1_lo'] = A_lo[0::2], A_lo[1::2]
    out['Be1_hi'], out['Bo1_hi'] = A_hi[0::2], A_hi[1::2]
    # zero-top variants: lhsT base partitions are limited to {0,32,64}, so
    # the 4th 32-row band slot (base 96) runs as K=64 at base 64 with the
    # top half of the matrix zeroed.
    z32 = np.zeros((32, 128))
    for nm in ('Be3_lo', 'Bo3_lo', 'Be3_hi', 'Bo3_hi'):
        out[nm + 'Z'] = np.vstack([z32, out[nm]])
    return {k: np.ascontiguousarray(v, np.float16) for k, v in out.items()}


MAT_SHAPES = {
    'M3_lo': (64, 128),
    'L3_hi_R': (64, 256), 'L3_hi_I': (64, 256),
    'L3_q_R': (128, 256), 'L3_q_I': (128, 256),
    'Be3_lo': (32, 128), 'Bo3_lo': (32, 128),
    'Be3_hi': (32, 128), 'Bo3_hi': (32, 128),
    'Be3_loZ': (64, 128), 'Bo3_loZ': (64, 128),
    'Be3_hiZ': (64, 128), 'Bo3_hiZ': (64, 128),
    'M2_lo': (128, 256),
    'L2_hi_R': (128, 512), 'L2_hi_I': (128, 512),
    'L2_lo_R': (128, 512), 'L2_lo_I': (128, 512),
    'Be2_lo': (64, 256), 'Bo2_lo': (64, 256),
    'Be2_hi': (64, 256), 'Bo2_hi': (64, 256),
    'Alo_a': (128, 256), 'Alo_b': (128, 256),
    'L1hi_w1r': (128, 512), 'L1hi_w2r': (128, 512),
    'L1hi_w1i': (128, 512), 'L1hi_w2i': (128, 512),
    'L1lo_w1r': (128, 512), 'L1lo_w2r': (128, 512),
    'L1lo_w1i': (128, 512), 'L1lo_w2i': (128, 512),
    'Be1_lo': (128, 256), 'Bo1_lo': (128, 256),
    'Be1_hi': (128, 256), 'Bo1_hi': (128, 256),
}

BLOB_A0 = ['M3_lo', 'L3_hi_R', 'L3_hi_I', 'L3_q_R', 'L3_q_I']
BLOB_A1 = ['Be3_lo', 'Bo3_lo', 'Be3_hi', 'Bo3_hi',
           'Be3_loZ', 'Bo3_loZ', 'Be3_hiZ', 'Bo3_hiZ',
           'M2_lo', 'L2_hi_R', 'L2_hi_I', 'L2_lo_R', 'L2_lo_I',
           'Be2_lo', 'Bo2_lo', 'Be2_hi', 'Bo2_hi']
BLOB_B = ['Alo_a', 'Alo_b',
          'L1hi_w1r', 'L1hi_w2r', 'L1hi_w1i', 'L1hi_w2i',
          'L1lo_w1r', 'L1lo_w2r', 'L1lo_w1i', 'L1lo_w2i',
          'Be1_lo', 'Bo1_lo', 'Be1_hi', 'Bo1_hi']
BLOB_A0_COLS = sum(MAT_SHAPES[n][1] for n in BLOB_A0)
BLOB_A1_COLS = sum(MAT_SHAPES[n][1] for n in BLOB_A1)
BLOB_B_COLS = sum(MAT_SHAPES[n][1] for n in BLOB_B)


def pack_blobs(mats):
    def pack(names, cols):
        blob = np.zeros((128, cols), np.float16)
        c = 0
        for n in names:
            K, N = MAT_SHAPES[n]
            for r in range(128 // K):  # replicate K<128 mats across parts
                blob[r * K:(r + 1) * K, c:c + N] = mats[n]
            c += N
        return blob
    return (pack(BLOB_A0, BLOB_A0_COLS), pack(BLOB_A1, BLOB_A1_COLS),
            pack(BLOB_B, BLOB_B_COLS))


# ---------------------------------------------------------------------------
# Bass kernel
# ---------------------------------------------------------------------------
def split_excess_waits(nc, max_waits=1):
    """walrus CTRL codegen allows only one sem wait per instruction; move
    excess waits onto NoOps inserted just before the offending instruction."""
    ctr = 0
    for fn in nc.m.functions:
        for bb in fn.blocks:
            new_list = []
            for inst in bb.instructions:
                si = inst.sync_info
                if si is not None and si.on_wait and len(si.on_wait) > max_waits:
                    waits = list(si.on_wait)
                    keep, extra = waits[:max_waits], waits[max_waits:]
                    for i in range(0, len(extra), max_waits):
                        nop = mybir.InstNoOp(
                            name=f"wait_split_{ctr}", ins=[], outs=[])
                        ctr += 1
                        nop.engine = inst.engine
                        nop.sync_info = mybir.SyncInfo(
                            on_wait=extra[i:i + max_waits], on_update=[])
                        nc.register_instruction(nop)
                        new_list.append(nop)
                    inst.sync_info = mybir.SyncInfo(
                        on_wait=keep,
                        on_update=list(si.on_update) if si.on_update else [])
                new_list.append(inst)
            bb.instructions[:] = new_list
    return ctr


def build_nc():
    nc = bass.Bass()
    yl_d = nc.dram_tensor("yl", [IMGS_PER_CORE, 64, 64], F16,
                          kind="ExternalInput")
    yh2_d = nc.dram_tensor("yh2", [IMGS_PER_CORE, 6, 32, 32, 2], F16,
                           kind="ExternalInput")
    yh1_d = nc.dram_tensor("yh1", [IMGS_PER_CORE, 6, 64, 64, 2], F16,
                           kind="ExternalInput")
    yh0_d = nc.dram_tensor("yh0", [IMGS_PER_CORE, 6, 128, 128, 2], F16,
                           kind="ExternalInput")
    out_d = nc.dram_tensor("out", [IMGS_PER_CORE, 256, 256], F16,
                           kind="ExternalOutput")
    matsA0_d = nc.dram_tensor("matsA0", [128, BLOB_A0_COLS], F16,
                              kind="ExternalInput")
    matsA1_d = nc.dram_tensor("matsA1", [128, BLOB_A1_COLS], F16,
                              kind="ExternalInput")
    matsB_d = nc.dram_tensor("matsB", [128, BLOB_B_COLS], F16,
                             kind="ExternalInput")

    with TileContext(nc) as tc:
        with tc.tile_pool(name="mats", bufs=1) as matpool, \
             tc.tile_pool(name="ins", bufs=1) as inpool, \
             tc.tile_pool(name="zs", bufs=1) as zpool, \
             tc.tile_pool(name="mid", bufs=3) as midpool, \
             tc.tile_pool(name="outp", bufs=4) as outpool:

            # --- matrix blobs: separate tiles so deps stay fine-grained;
            # the tiny L3-col blob lands first so img0 starts early ---
            blobA0_t = matpool.tile([128, BLOB_A0_COLS], F16, tag="blobA0")
            nc.scalar.dma_start(out=blobA0_t[:], in_=matsA0_d[:])
            blobA1_t = matpool.tile([128, BLOB_A1_COLS], F16, tag="blobA1")
            nc.scalar.dma_start(out=blobA1_t[:], in_=matsA1_d[:])
            blobB_t = matpool.tile([128, BLOB_B_COLS], F16, tag="blobB")
            nc.sync.dma_start(out=blobB_t[:], in_=matsB_d[:])
            mats = {}
            mat_loc = {}
            for blob_t, names in ((blobA0_t, BLOB_A0), (blobA1_t, BLOB_A1),
                                  (blobB_t, BLOB_B)):
                c = 0
                for n in names:
                    K, N = MAT_SHAPES[n]
                    mats[n] = blob_t[0:K, c:c + N]
                    mat_loc[n] = (blob_t, c)
                    c += N

            def mat_at(name, poff):
                blob, c = mat_loc[name]
                K, N = MAT_SHAPES[name]
                return blob[poff:poff + K, c:c + N]

            # --- batched input sweeps ---
            # z3all[h, (i w)] <- yl[i, h, w]
            z3all = inpool.tile([64, 16 * 64], F16, tag="z3all")
            nc.gpsimd.dma_start(
                out=z3all.rearrange("h (i x) -> h i x", i=16),
                in_=yl_d.rearrange("i h x -> h i x"))
            # per-orientation all-image sweeps (DMA APs max 3 dims)
            def band_sweep(tile_ap, p0, p1, src5, i=16):
                nc.gpsimd.dma_start(
                    out=tile_ap[p0:p1, :].rearrange("h (i x) -> h i x", i=i),
                    in_=src5.rearrange("i h w r -> h i (w r)"))
            lh3all = inpool.tile([64, 16 * 64], F16, tag="lh3all")
            band_sweep(lh3all, 0, 32, yh2_d[:, 0])
            band_sweep(lh3all, 32, 64, yh2_d[:, 5])
            # q3all: parts 0:64 = orient pair (2,3), 64:128 = (1,4)
            q3all = inpool.tile([128, 16 * 64], F16, tag="q3all")
            band_sweep(q3all, 0, 32, yh2_d[:, 2])
            band_sweep(q3all, 32, 64, yh2_d[:, 3])
            band_sweep(q3all, 64, 96, yh2_d[:, 1])
            band_sweep(q3all, 96, 128, yh2_d[:, 4])
            # yh1 band pair tiles, one DMA per orientation
            lh2all = inpool.tile([128, 16 * 128], F16, tag="lh2all")
            band_sweep(lh2all, 0, 64, yh1_d[:, 0])
            band_sweep(lh2all, 64, 128, yh1_d[:, 5])
            hl2all = inpool.tile([128, 16 * 128], F16, tag="hl2all")
            band_sweep(hl2all, 0, 64, yh1_d[:, 2])
            band_sweep(hl2all, 64, 128, yh1_d[:, 3])
            hh2all = inpool.tile([128, 16 * 128], F16, tag="hh2all")
            band_sweep(hh2all, 0, 64, yh1_d[:, 1])
            band_sweep(hh2all, 64, 128, yh1_d[:, 4])
            # yh0: 4 groups of 4 imgs on the sync queue, separate tiles
            yh0g = {}
            for g in range(4):
                t = inpool.tile([128, 4 * 1536], F16, tag=f"yh0g{g}",
                                name=f"yh0g{g}")
                nc.sync.dma_start(
                    out=t.rearrange("h (g x) -> h g x", g=24),
                    in_=yh0_d[4 * g:4 * g + 4].rearrange(
                        "i o h w r -> h (i o) (w r)"))
                yh0g[g] = t

            z2s = {img: zpool.tile([128, 128], F16, tag=f"z2_{img}",
                                   name=f"z2_{img}")
                   for img in range(IMGS_PER_CORE)}
            z1s = {img: zpool.tile([128, 512], F16, tag=f"z1_{img}",
                                   name=f"z1_{img}")
                   for img in range(IMGS_PER_CORE)}

            def mm(out_ap, lhsT, rhs_name, start, stop, poff=0):
                rhs = mats[rhs_name] if poff == 0 else mat_at(rhs_name, poff)
                nc.tensor.matmul(out_ap, lhsT, rhs, start=start, stop=stop)

            # ===========================================================
            # Phase L3: quad-packed (M=128 across 4 imgs) col stages,
            # per-img row stages via lhsT partition offsets
            # ===========================================================
            with tc.tile_pool(name="ps3c", bufs=2, space="PSUM") as ps3cpool,\
                 tc.tile_pool(name="ps3r", bufs=2, space="PSUM") as ps3rpool:
                for g in range(4):
                    base = 4 * g
                    c0 = base * 64
                    p3 = ps3cpool.tile([128, 1024], F32, tag="p3")
                    mm(p3[:, 0:128], z3all[:, c0:c0 + 128], 'M3_lo',
                       True, True)
                    mm(p3[:, 128:256], z3all[:, c0 + 128:c0 + 256], 'M3_lo',
                       True, True)
                    lq = lh3all[:, c0:c0 + 256]
                    mm(p3[:, 256:512], lq[:, 0::2], 'L3_hi_R', True, False)
                    mm(p3[:, 256:512], lq[:, 1::2], 'L3_hi_I', False, True)
                    qq = q3all[:, c0:c0 + 256]
                    mm(p3[:, 512:768], qq[:, 0::2], 'L3_q_R', True, False)
                    mm(p3[:, 512:768], qq[:, 1::2], 'L3_q_I', False, True)
                    y1z_s = midpool.tile([128, 256], F16, tag="y1z3")
                    nc.scalar.copy(y1z_s[:], p3[:, 0:256])
                    y1b_s = midpool.tile([128, 256], F16, tag="y1b3")
                    nc.vector.tensor_copy(out=y1b_s[:], in_=p3[:, 256:512])
                    y2b_s = midpool.tile([128, 256], F16, tag="y2b3")
                    nc.vector.tensor_copy(out=y2b_s[:], in_=p3[:, 512:768])

                    prow = ps3rpool.tile([128, 512], F32, tag="p3r")
                    for i in range(4):
                        img = base + i
                        zp = prow[:, i * 128:(i + 1) * 128]
                        zoff = (i % 2) * 64
                        zcol = (i // 2) * 128
                        mm(zp, y1z_s[zoff:zoff + 64, zcol:zcol + 128],
                           'M3_lo', True, False, poff=zoff)
                        if i < 3:
                            boff, sfx, bk = i * 32, '', 32
                        else:
                            boff, sfx, bk = 64, 'Z', 64
                        mm(zp, y1b_s[boff:boff + bk, 0:128],
                           'Be3_lo' + sfx, False, False, poff=boff)
                        mm(zp, y1b_s[boff:boff + bk, 128:256],
                           'Bo3_lo' + sfx, False, False, poff=boff)
                        mm(zp, y2b_s[boff:boff + bk, 0:128],
                           'Be3_hi' + sfx, False, False, poff=boff)
                        mm(zp, y2b_s[boff:boff + bk, 128:256],
                           'Bo3_hi' + sfx, False, True, poff=boff)
                        if i % 2 == 0:
                            nc.scalar.copy(z2s[img][:], zp)
                        else:
                            nc.vector.tensor_copy(out=z2s[img][:], in_=zp)

            # ===========================================================
            # Phase L2: pair-packed band col stages (M=128 across 2 imgs),
            # per-img row stages via lhsT partition offsets
            # ===========================================================
            with tc.tile_pool(name="ps2c", bufs=3, space="PSUM") as ps2cpool,\
                 tc.tile_pool(name="ps2r", bufs=2, space="PSUM") as ps2rpool:
                for p in range(8):
                    a, b = 2 * p, 2 * p + 1
                    cc = p * 256
                    pA = ps2cpool.tile([128, 512], F32, tag="p2")
                    mm(pA[:, 0:256], z2s[a][:], 'M2_lo', True, True)
                    mm(pA[:, 256:512], z2s[b][:], 'M2_lo', True, True)
                    pB = ps2cpool.tile([128, 512], F32, tag="p2")
                    lp = lh2all[:, cc:cc + 256]
                    mm(pB[:], lp[:, 0::2], 'L2_hi_R', True, False)
                    mm(pB[:], lp[:, 1::2], 'L2_hi_I', False, True)
                    pC = ps2cpool.tile([128, 512], F32, tag="p2")
                    hp = hl2all[:, cc:cc + 256]
                    hq = hh2all[:, cc:cc + 256]
                    mm(pC[:], hp[:, 0::2], 'L2_lo_R', True, False)
                    mm(pC[:], hp[:, 1::2], 'L2_lo_I', False, False)
                    mm(pC[:], hq[:, 0::2], 'L2_hi_R', False, False)
                    mm(pC[:], hq[:, 1::2], 'L2_hi_I', False, True)
                    y1zT_s = midpool.tile([128, 512], F16, tag="y1zT2")
                    nc.scalar.copy(y1zT_s[:], pA[:])
                    b1_s = midpool.tile([128, 512], F16, tag="b1_2")
                    nc.vector.tensor_copy(out=b1_s[:], in_=pB[:])
                    b2_s = midpool.tile([128, 512], F16, tag="b2_2")
                    nc.vector.tensor_copy(out=b2_s[:], in_=pC[:])

                    for i, img in enumerate((a, b)):
                        off = i * 64
                        abase = i * 256
                        p2r = ps2rpool.tile([128, 512], F32, tag="p2r")
                        for m in range(2):
                            zc = p2r[:, m * 256:(m + 1) * 256]
                            msl = slice(m * 128, (m + 1) * 128)
                            osl = slice(256 + m * 128, 256 + (m + 1) * 128)
                            mm(zc, y1zT_s[:, abase + m * 128:
                                          abase + (m + 1) * 128],
                               'M2_lo', True, False)
                            mm(zc, b1_s[off:off + 64, msl], 'Be2_lo',
                               False, False, poff=off)
                            mm(zc, b1_s[off:off + 64, osl], 'Bo2_lo',
                               False, False, poff=off)
                            mm(zc, b2_s[off:off + 64, msl], 'Be2_hi',
                               False, False, poff=off)
                            mm(zc, b2_s[off:off + 64, osl], 'Bo2_hi',
                               False, True, poff=off)
                            if m == 0:
                                nc.scalar.copy(z1s[img][:, 0:256], zc)
                            else:
                                nc.vector.tensor_copy(
                                    out=z1s[img][:, 256:512], in_=zc)

            # ===========================================================
            # Phase L1: z1 [256,256] + yh0 bands -> out [256,256], all imgs
            # ===========================================================
            with tc.tile_pool(name="ps1c", bufs=3, space="PSUM") as ps1cpool,\
                 tc.tile_pool(name="ps1r", bufs=2, space="PSUM") as ps1rpool:
                for img in range(IMGS_PER_CORE):
                    yh0t = yh0g[img // 4]
                    ib = (img % 4) * 1536
                    o_t = {o: yh0t[:, ib + o * 256:ib + (o + 1) * 256]
                           for o in range(6)}
                    z1_s = z1s[img]
                    # phase A: y1 = band + lowpass, merged in w-polyphase
                    # layout [E(h 256) | O(h 256)]  (partitions = w')
                    p1a = ps1cpool.tile([128, 512], F32, tag="p1")
                    y1_p = p1a[:]
                    mm(y1_p, o_t[0][:, 0::2], 'L1hi_w1r', True, False)
                    mm(y1_p, o_t[5][:, 0::2], 'L1hi_w2r', False, False)
                    mm(y1_p, o_t[0][:, 1::2], 'L1hi_w1i', False, False)
                    mm(y1_p, o_t[5][:, 1::2], 'L1hi_w2i', False, False)
                    mm(p1a[:, 0:256], z1_s[:, 0:256:2], 'Alo_a',
                       False, False)
                    mm(p1a[:, 0:256], z1_s[:, 256:512:2], 'Alo_b',
                       False, True)
                    mm(p1a[:, 256:512], z1_s[:, 1:256:2], 'Alo_a',
                       False, False)
                    mm(p1a[:, 256:512], z1_s[:, 257:512:2], 'Alo_b',
                       False, True)
                    y1_s = midpool.tile([128, 512], F16, tag="y1m")
                    nc.vector.tensor_copy(out=y1_s[:], in_=y1_p)

                    # phase B: y2b e|o [0:512)
                    p1b = ps1cpool.tile([128, 512], F32, tag="p1")
                    y2b_p = p1b[:]
                    mm(y2b_p, o_t[2][:, 0::2], 'L1lo_w1r', True, False)
                    mm(y2b_p, o_t[3][:, 0::2], 'L1lo_w2r', False, False)
                    mm(y2b_p, o_t[2][:, 1::2], 'L1lo_w1i', False, False)
                    mm(y2b_p, o_t[3][:, 1::2], 'L1lo_w2i', False, False)
                    mm(y2b_p, o_t[1][:, 0::2], 'L1hi_w1r', False, False)
                    mm(y2b_p, o_t[4][:, 0::2], 'L1hi_w2r', False, False)
                    mm(y2b_p, o_t[1][:, 1::2], 'L1hi_w1i', False, False)
                    mm(y2b_p, o_t[4][:, 1::2], 'L1hi_w2i', False, True)
                    y2b1_s = midpool.tile([128, 512], F16, tag="y2b1")
                    nc.vector.tensor_copy(out=y2b1_s[:], in_=y2b_p)

                    # row stage -> out [256, 256] in two h-chunks; single
                    # store DMA per image ([a p] x <- p [a x])
                    p1r = ps1rpool.tile([128, 512], F32, tag="p1r")
                    ot = outpool.tile([128, 512], F16, tag="ot")
                    for m in range(2):
                        oc = p1r[:, m * 256:(m + 1) * 256]
                        msl = slice(m * 128, (m + 1) * 128)
                        osl = slice(256 + m * 128, 256 + (m + 1) * 128)
                        mm(oc, y1_s[:, msl], 'Be1_lo', True, False)
                        mm(oc, y1_s[:, osl], 'Bo1_lo', False, False)
                        mm(oc, y2b1_s[:, msl], 'Be1_hi', False, False)
                        mm(oc, y2b1_s[:, osl], 'Bo1_hi', False, True)
                        if m == 0:
                            nc.scalar.copy(ot[:, 0:256], oc)
                        else:
                            nc.vector.tensor_copy(out=ot[:, 256:512], in_=oc)
                    nc.gpsimd.dma_start(
                        out=out_d[img].rearrange("(a p) x -> p a x", a=2),
                        in_=ot.rearrange("p (a x) -> p a x", a=2))

    split_excess_waits(nc)
    return nc


# ---------------------------------------------------------------------------
# Entry point
# ---------------------------------------------------------------------------
_NC_CACHE = []
_LAST_RESULT = []  # last BassKernelResults (exec_time_ns when BASS_TRACE=1)


def _axon_reset():
    try:
        import ctypes
        lib = ctypes.CDLL('/opt/axon/libaxon_pjrt.so')
        lib.axon_reset.restype = ctypes.c_int64
        lib.axon_reset()
    except Exception:
        pass


def kernel(yl, yh0, yh1, yh2, g0o, g1o, g0a, g0b, g1a, g1b):
    yl = np.ascontiguousarray(np.asarray(yl, np.float16))
    yh0 = np.ascontiguousarray(np.asarray(yh0, np.float16))
    yh1 = np.ascontiguousarray(np.asarray(yh1, np.float16))
    yh2 = np.ascontiguousarray(np.asarray(yh2, np.float16))
    assert yl.shape == (8, 16, 64, 64)

    mats = build_matrices(g0o, g1o, g0a, g0b, g1a, g1b)
    blobA0, blobA1, blobB = pack_blobs(mats)
    if not _NC_CACHE:
        _NC_CACHE.append(build_nc())
    nc = _NC_CACHE[0]

    in_maps = []
    for core in range(N_CORES):
        m = {"yl": yl[core], "yh0": yh0[core],
             "yh1": yh1[core], "yh2": yh2[core],
             "matsA0": blobA0, "matsA1": blobA1, "matsB": blobB}
        in_maps.append(m)

    try:
        res = run_bass_kernel_spmd(nc, in_maps, list(range(N_CORES)))
    except Exception as e:  # wedged exec unit: reset the axon device, retry
        if "UNAVAILABLE" not in str(e) and "unrecoverable" not in str(e):
            raise
        _axon_reset()
        res = run_bass_kernel_spmd(nc, in_maps, list(range(N_CORES)))
    _LAST_RESULT.clear()
    _LAST_RESULT.append(res)
    out = np.stack([res.results[i]["out"] for i in range(N_CORES)], axis=0)
    return np.ascontiguousarray(out.astype(np.float32))


# revision 24
# speedup vs baseline: 1.2584x; 1.0282x over previous
"""DTCWT 3-level inverse on 8 Trainium2 NeuronCores.

Every filtering stage is a banded matmul on the tensor engine in fp16
(PSUM accumulates fp32; ~7e-4 total rel err vs the 2e-2 gate).

All stages use "data as lhsT" mode: matmul(out, lhsT=data[K=h, M=w],
rhs=mat[K=h, N=h_out]) contracts over the partition dim of the data and
yields the filtered image TRANSPOSED ([w, h_out]); column and row stages
then alternate orientation naturally with zero explicit transposes.

The c2q band construction is folded into the matrices; at L1 the lowpass
path is additionally merged into the band polyphase layout ([E|O] w-planes)
so the final row stage is 4 accumulation passes instead of 6.

Schedule: phase-major (L3 x16, L2 x16, L1 x16) with double/triple-buffered
PSUM pools so neighbouring images' matmuls hide each other's copy latency.
DMA queue slots cost ~600ns regardless of size, so all loads are batched
into a handful of giant multi-dim DMAs (2 matrix blobs, 9 input sweeps,
1 store per image).

Sharding: pure data parallel over batch N (8 cores x 16 channels each).
"""
import sys

for _p in ('/opt/trn_rl_repo',):
    if _p not in sys.path:
        sys.path.append(_p)

import numpy as np
import concourse.bass as bass
import concourse.mybir as mybir
from concourse.tile import TileContext
from concourse.bass_utils import run_bass_kernel_spmd

SQRT_HALF = 0.7071067811865476
N_CORES = 8
IMGS_PER_CORE = 16
F32 = mybir.dt.float32
F16 = mybir.dt.float16


# ---------------------------------------------------------------------------
# Host-side matrix construction (numpy, float64)
# ---------------------------------------------------------------------------
def _conv_rows_valid(x, h):
    hr = h[::-1]
    taps = h.shape[0]
    n = x.shape[-2] - taps + 1
    out = hr[0] * x[..., 0:n, :]
    for k in range(1, taps):
        out = out + hr[k] * x[..., k:k + n, :]
    return out


def _pad_rows_symmetric(x, m):
    pad = [(0, 0)] * (x.ndim - 2) + [(m, m), (0, 0)]
    return np.pad(x, pad, mode='symmetric')


def _colfilter(x, h):
    return _conv_rows_valid(_pad_rows_symmetric(x, h.shape[0] // 2), h)


def _colifilt(x, ha, hb, highpass):
    m = ha.shape[0]
    m2 = m // 2
    r = x.shape[-2]
    xp = _pad_rows_symmetric(x, m2)
    xe = xp[..., 1:r + m - 2:2, :]
    xo = xp[..., 2:r + m - 1:2, :]
    xa, xb = (xe, xo) if highpass else (xo, xe)
    hao, hae = ha[0::2], ha[1::2]
    hbo, hbe = hb[0::2], hb[1::2]
    y0 = _conv_rows_valid(xb, hao)
    y1 = _conv_rows_valid(xa, hbo)
    y2 = _conv_rows_valid(xb, hae)
    y3 = _conv_rows_valid(xa, hbe)
    y = np.stack([y0, y1, y2, y3], axis=-2)
    return y.reshape(y.shape[:-3] + (2 * r, y.shape[-1]))


def _op_matrix(op, n):
    """M[h_in, h_out] with out[h_out, w] = sum_h M[h, h_out] x[h, w]."""
    return np.ascontiguousarray(op(np.eye(n, dtype=np.float64)).T)


def build_matrices(g0o, g1o, g0a, g0b, g1a, g1b):
    """All device matrices as {name: fp16 ndarray}."""
    g0o = np.asarray(g0o, np.float64)
    g1o = np.asarray(g1o, np.float64)
    g0a = np.asarray(g0a, np.float64)
    g0b = np.asarray(g0b, np.float64)
    g1a = np.asarray(g1a, np.float64)
    g1b = np.asarray(g1b, np.float64)
    s = SQRT_HALF
    hs, vs = np.hstack, np.vstack
    out = {}

    def upsample_level(R, tag):
        Mlo = _op_matrix(lambda x: _colifilt(x, g0b, g0a, False), R)  # [R, 2R]
        Mhi = _op_matrix(lambda x: _colifilt(x, g1b, g1a, True), R)
        Me_h, Mo_h = s * Mhi[0::2], s * Mhi[1::2]                     # [R/2, 2R]
        Me_l, Mo_l = s * Mlo[0::2], s * Mlo[1::2]
        out[f'M{tag}_lo'] = Mlo
        # pair-stacked [w1; w2] col rhs, e|o column-concatenated
        #   e: w1r*Me + w2r*Me + w1i*Mo - w2i*Mo
        #   o: -w1r*Mo + w2r*Mo + w1i*Me + w2i*Me
        out[f'L{tag}_hi_R'] = hs([vs([Me_h, Me_h]), vs([-Mo_h, Mo_h])])
        out[f'L{tag}_hi_I'] = hs([vs([Mo_h, -Mo_h]), vs([Me_h, Me_h])])
        out[f'L{tag}_lo_R'] = hs([vs([Me_l, Me_l]), vs([-Mo_l, Mo_l])])
        out[f'L{tag}_lo_I'] = hs([vs([Mo_l, -Mo_l]), vs([Me_l, Me_l])])
        # row stage (polyphase-column recombination)
        out[f'Be{tag}_lo'], out[f'Bo{tag}_lo'] = Mlo[0::2], Mlo[1::2]
        out[f'Be{tag}_hi'], out[f'Bo{tag}_hi'] = Mhi[0::2], Mhi[1::2]

    upsample_level(64, '3')
    upsample_level(128, '2')
    # L3 quad stacks: [hl pair (lo mats); hh pair (hi mats)], K=128
    out['L3_q_R'] = vs([out['L3_lo_R'], out['L3_hi_R']])
    out['L3_q_I'] = vs([out['L3_lo_I'], out['L3_hi_I']])
    del out['L3_lo_R'], out['L3_lo_I']  # only used inside the quad at L3

    # L1 (colfilter, size-preserving, n=256)
    A_lo = _op_matrix(lambda x: _colfilter(x, g0o), 256)              # [256, 256]
    A_hi = _op_matrix(lambda x: _colfilter(x, g1o), 256)
    out['Alo_a'], out['Alo_b'] = A_lo[0:128], A_lo[128:256]
    for x, A in (('hi', A_hi), ('lo', A_lo)):
        Me, Mo = s * A[0::2], s * A[1::2]                             # [128, 256]
        out[f'L1{x}_w1r'] = hs([Me, -Mo])
        out[f'L1{x}_w2r'] = hs([Me, Mo])
        out[f'L1{x}_w1i'] = hs([Mo, Me])
        out[f'L1{x}_w2i'] = hs([-Mo, Me])
    out['Be1_lo'], out['Bo1_lo'] = A_lo[0::2], A_lo[1::2]
    out['Be1_hi'], out['Bo1_hi'] = A_hi[0::2], A_hi[1::2]
    # zero-top variants: lhsT base partitions are limited to {0,32,64}, so
    # the 4th 32-row band slot (base 96) runs as K=64 at base 64 with the
    # top half of the matrix zeroed.
    z32 = np.zeros((32, 128))
    for nm in ('Be3_lo', 'Bo3_lo', 'Be3_hi', 'Bo3_hi'):
        out[nm + 'Z'] = np.vstack([z32, out[nm]])
    return {k: np.ascontiguousarray(v, np.float16) for k, v in out.items()}


MAT_SHAPES = {
    'M3_lo': (64, 128),
    'L3_hi_R': (64, 256), 'L3_hi_I': (64, 256),
    'L3_q_R': (128, 256), 'L3_q_I': (128, 256),
    'Be3_lo': (32, 128), 'Bo3_lo': (32, 128),
    'Be3_hi': (32, 128), 'Bo3_hi': (32, 128),
    'Be3_loZ': (64, 128), 'Bo3_loZ': (64, 128),
    'Be3_hiZ': (64, 128), 'Bo3_hiZ': (64, 128),
    'M2_lo': (128, 256),
    'L2_hi_R': (128, 512), 'L2_hi_I': (128, 512),
    'L2_lo_R': (128, 512), 'L2_lo_I': (128, 512),
    'Be2_lo': (64, 256), 'Bo2_lo': (64, 256),
    'Be2_hi': (64, 256), 'Bo2_hi': (64, 256),
    'Alo_a': (128, 256), 'Alo_b': (128, 256),
    'L1hi_w1r': (128, 512), 'L1hi_w2r': (128, 512),
    'L1hi_w1i': (128, 512), 'L1hi_w2i': (128, 512),
    'L1lo_w1r': (128, 512), 'L1lo_w2r': (128, 512),
    'L1lo_w1i': (128, 512), 'L1lo_w2i': (128, 512),
    'Be1_lo': (128, 256), 'Bo1_lo': (128, 256),
    'Be1_hi': (128, 256), 'Bo1_hi': (128, 256),
}

BLOB_A0 = ['M3_lo', 'L3_hi_R', 'L3_hi_I', 'L3_q_R', 'L3_q_I',
           'Be3_lo', 'Bo3_lo', 'Be3_hi', 'Bo3_hi',
           'Be3_loZ', 'Bo3_loZ', 'Be3_hiZ', 'Bo3_hiZ']
BLOB_A1 = ['M2_lo', 'L2_hi_R', 'L2_hi_I', 'L2_lo_R', 'L2_lo_I',
           'Be2_lo', 'Bo2_lo', 'Be2_hi', 'Bo2_hi']
BLOB_B = ['Alo_a', 'Alo_b',
          'L1hi_w1r', 'L1hi_w2r', 'L1hi_w1i', 'L1hi_w2i',
          'L1lo_w1r', 'L1lo_w2r', 'L1lo_w1i', 'L1lo_w2i',
          'Be1_lo', 'Bo1_lo', 'Be1_hi', 'Bo1_hi']
BLOB_A0_COLS = sum(MAT_SHAPES[n][1] for n in BLOB_A0)
BLOB_A1_COLS = sum(MAT_SHAPES[n][1] for n in BLOB_A1)
BLOB_B_COLS = sum(MAT_SHAPES[n][1] for n in BLOB_B)


def pack_blobs(mats):
    def pack(names, cols):
        blob = np.zeros((128, cols), np.float16)
        c = 0
        for n in names:
            K, N = MAT_SHAPES[n]
            for r in range(128 // K):  # replicate K<128 mats across parts
                blob[r * K:(r + 1) * K, c:c + N] = mats[n]
            c += N
        return blob
    return (pack(BLOB_A0, BLOB_A0_COLS), pack(BLOB_A1, BLOB_A1_COLS),
            pack(BLOB_B, BLOB_B_COLS))


# ---------------------------------------------------------------------------
# Bass kernel
# ---------------------------------------------------------------------------
def split_excess_waits(nc, max_waits=1):
    """walrus CTRL codegen allows only one sem wait per instruction; move
    excess waits onto NoOps inserted just before the offending instruction."""
    ctr = 0
    for fn in nc.m.functions:
        for bb in fn.blocks:
            new_list = []
            for inst in bb.instructions:
                si = inst.sync_info
                if si is not None and si.on_wait and len(si.on_wait) > max_waits:
                    waits = list(si.on_wait)
                    keep, extra = waits[:max_waits], waits[max_waits:]
                    for i in range(0, len(extra), max_waits):
                        nop = mybir.InstNoOp(
                            name=f"wait_split_{ctr}", ins=[], outs=[])
                        ctr += 1
                        nop.engine = inst.engine
                        nop.sync_info = mybir.SyncInfo(
                            on_wait=extra[i:i + max_waits], on_update=[])
                        nc.register_instruction(nop)
                        new_list.append(nop)
                    inst.sync_info = mybir.SyncInfo(
                        on_wait=keep,
                        on_update=list(si.on_update) if si.on_update else [])
                new_list.append(inst)
            bb.instructions[:] = new_list
    return ctr


def build_nc():
    nc = bass.Bass()
    yl_d = nc.dram_tensor("yl", [IMGS_PER_CORE, 64, 64], F16,
                          kind="ExternalInput")
    yh2_d = nc.dram_tensor("yh2", [IMGS_PER_CORE, 6, 32, 32, 2], F16,
                           kind="ExternalInput")
    yh1_d = nc.dram_tensor("yh1", [IMGS_PER_CORE, 6, 64, 64, 2], F16,
                           kind="ExternalInput")
    yh0_d = nc.dram_tensor("yh0", [IMGS_PER_CORE, 6, 128, 128, 2], F16,
                           kind="ExternalInput")
    out_d = nc.dram_tensor("out", [IMGS_PER_CORE, 256, 256], F16,
                           kind="ExternalOutput")
    matsA0_d = nc.dram_tensor("matsA0", [128, BLOB_A0_COLS], F16,
                              kind="ExternalInput")
    matsA1_d = nc.dram_tensor("matsA1", [128, BLOB_A1_COLS], F16,
                              kind="ExternalInput")
    matsB_d = nc.dram_tensor("matsB", [128, BLOB_B_COLS], F16,
                             kind="ExternalInput")

    with TileContext(nc) as tc:
        with tc.tile_pool(name="mats", bufs=1) as matpool, \
             tc.tile_pool(name="ins", bufs=1) as inpool, \
             tc.tile_pool(name="zs", bufs=1) as zpool, \
             tc.tile_pool(name="mid", bufs=3) as midpool, \
             tc.tile_pool(name="outp", bufs=4) as outpool:

            # --- matrix blobs: separate tiles so deps stay fine-grained;
            # the tiny L3-col blob lands first so img0 starts early ---
            blobA0_t = matpool.tile([128, BLOB_A0_COLS], F16, tag="blobA0")
            nc.scalar.dma_start(out=blobA0_t[:], in_=matsA0_d[:])
            blobA1_t = matpool.tile([128, BLOB_A1_COLS], F16, tag="blobA1")
            nc.scalar.dma_start(out=blobA1_t[:], in_=matsA1_d[:])
            blobB_t = matpool.tile([128, BLOB_B_COLS], F16, tag="blobB")
            nc.scalar.dma_start(out=blobB_t[:], in_=matsB_d[:])
            mats = {}
            mat_loc = {}
            for blob_t, names in ((blobA0_t, BLOB_A0), (blobA1_t, BLOB_A1),
                                  (blobB_t, BLOB_B)):
                c = 0
                for n in names:
                    K, N = MAT_SHAPES[n]
                    mats[n] = blob_t[0:K, c:c + N]
                    mat_loc[n] = (blob_t, c)
                    c += N

            def mat_at(name, poff):
                blob, c = mat_loc[name]
                K, N = MAT_SHAPES[name]
                return blob[poff:poff + K, c:c + N]

            # --- batched input sweeps ---
            # z3all[h, (i w)] <- yl[i, h, w]
            z3all = inpool.tile([64, 16 * 64], F16, tag="z3all")
            nc.gpsimd.dma_start(
                out=z3all.rearrange("h (i x) -> h i x", i=16),
                in_=yl_d.rearrange("i h x -> h i x"))
            # per-orientation all-image sweeps (DMA APs max 3 dims)
            def band_sweep(tile_ap, p0, p1, src5, i=16):
                nc.gpsimd.dma_start(
                    out=tile_ap[p0:p1, :].rearrange("h (i x) -> h i x", i=i),
                    in_=src5.rearrange("i h w r -> h i (w r)"))
            lh3all = inpool.tile([64, 16 * 64], F16, tag="lh3all")
            band_sweep(lh3all, 0, 32, yh2_d[:, 0])
            band_sweep(lh3all, 32, 64, yh2_d[:, 5])
            # q3all: parts 0:64 = orient pair (2,3), 64:128 = (1,4)
            q3all = inpool.tile([128, 16 * 64], F16, tag="q3all")
            band_sweep(q3all, 0, 32, yh2_d[:, 2])
            band_sweep(q3all, 32, 64, yh2_d[:, 3])
            band_sweep(q3all, 64, 96, yh2_d[:, 1])
            band_sweep(q3all, 96, 128, yh2_d[:, 4])
            # yh1 band pair tiles, one DMA per orientation
            lh2all = inpool.tile([128, 16 * 128], F16, tag="lh2all")
            band_sweep(lh2all, 0, 64, yh1_d[:, 0])
            band_sweep(lh2all, 64, 128, yh1_d[:, 5])
            hl2all = inpool.tile([128, 16 * 128], F16, tag="hl2all")
            band_sweep(hl2all, 0, 64, yh1_d[:, 2])
            band_sweep(hl2all, 64, 128, yh1_d[:, 3])
            hh2all = inpool.tile([128, 16 * 128], F16, tag="hh2all")
            band_sweep(hh2all, 0, 64, yh1_d[:, 1])
            band_sweep(hh2all, 64, 128, yh1_d[:, 4])
            # yh0: 4 groups of 4 imgs on the gpsimd queue (after yh1),
            # separate tiles so L1 deps stay per-group
            yh0g = {}
            for g in range(4):
                t = inpool.tile([128, 4 * 1536], F16, tag=f"yh0g{g}",
                                name=f"yh0g{g}")
                nc.gpsimd.dma_start(
                    out=t.rearrange("h (g x) -> h g x", g=24),
                    in_=yh0_d[4 * g:4 * g + 4].rearrange(
                        "i o h w r -> h (i o) (w r)"))
                yh0g[g] = t

            z2p = {p: zpool.tile([128, 256], F16, tag=f"z2p_{p}",
                                 name=f"z2p_{p}")
                   for p in range(IMGS_PER_CORE // 2)}
            z1p = {p: zpool.tile([128, 1024], F16, tag=f"z1p_{p}",
                                 name=f"z1p_{p}")
                   for p in range(IMGS_PER_CORE // 2)}

            def z2s_ap(img):
                return z2p[img // 2][:, (img % 2) * 128:(img % 2) * 128 + 128]

            def z1s_ap(img):
                return z1p[img // 2][:, (img % 2) * 512:(img % 2) * 512 + 512]

            def mm(out_ap, lhsT, rhs_name, start, stop, poff=0):
                rhs = mats[rhs_name] if poff == 0 else mat_at(rhs_name, poff)
                nc.tensor.matmul(out_ap, lhsT, rhs, start=start, stop=stop)

            # ===========================================================
            # Phase L3: quad-packed (M=128 across 4 imgs) col stages,
            # per-img row stages via lhsT partition offsets
            # ===========================================================
            with tc.tile_pool(name="ps3c", bufs=2, space="PSUM") as ps3cpool,\
                 tc.tile_pool(name="ps3r", bufs=2, space="PSUM") as ps3rpool:
                for g in range(4):
                    base = 4 * g
                    c0 = base * 64
                    p3 = ps3cpool.tile([128, 1024], F32, tag="p3")
                    mm(p3[:, 0:128], z3all[:, c0:c0 + 128], 'M3_lo',
                       True, True)
                    mm(p3[:, 128:256], z3all[:, c0 + 128:c0 + 256], 'M3_lo',
                       True, True)
                    lq = lh3all[:, c0:c0 + 256]
                    mm(p3[:, 256:512], lq[:, 0::2], 'L3_hi_R', True, False)
                    mm(p3[:, 256:512], lq[:, 1::2], 'L3_hi_I', False, True)
                    qq = q3all[:, c0:c0 + 256]
                    mm(p3[:, 512:768], qq[:, 0::2], 'L3_q_R', True, False)
                    mm(p3[:, 512:768], qq[:, 1::2], 'L3_q_I', False, True)
                    y1z_s = midpool.tile([128, 256], F16, tag="y1z3")
                    nc.scalar.copy(y1z_s[:], p3[:, 0:256])
                    y1b_s = midpool.tile([128, 256], F16, tag="y1b3")
                    nc.vector.tensor_copy(out=y1b_s[:], in_=p3[:, 256:512])
                    y2b_s = midpool.tile([128, 256], F16, tag="y2b3")
                    nc.vector.tensor_copy(out=y2b_s[:], in_=p3[:, 512:768])

                    prow = ps3rpool.tile([128, 512], F32, tag="p3r")
                    for i in range(4):
                        img = base + i
                        zp = prow[:, i * 128:(i + 1) * 128]
                        zoff = (i % 2) * 64
                        zcol = (i // 2) * 128
                        mm(zp, y1z_s[zoff:zoff + 64, zcol:zcol + 128],
                           'M3_lo', True, False, poff=zoff)
                        if i < 3:
                            boff, sfx, bk = i * 32, '', 32
                        else:
                            boff, sfx, bk = 64, 'Z', 64
                        mm(zp, y1b_s[boff:boff + bk, 0:128],
                           'Be3_lo' + sfx, False, False, poff=boff)
                        mm(zp, y1b_s[boff:boff + bk, 128:256],
                           'Bo3_lo' + sfx, False, False, poff=boff)
                        mm(zp, y2b_s[boff:boff + bk, 0:128],
                           'Be3_hi' + sfx, False, False, poff=boff)
                        mm(zp, y2b_s[boff:boff + bk, 128:256],
                           'Bo3_hi' + sfx, False, True, poff=boff)
                        if i % 2 == 0:
                            nc.scalar.copy(z2s_ap(img), zp)
                        else:
                            nc.vector.tensor_copy(out=z2s_ap(img), in_=zp)

            # ===========================================================
            # Phase L2: pair-packed band col stages (M=128 across 2 imgs),
            # per-img row stages via lhsT partition offsets
            # ===========================================================
            with tc.tile_pool(name="ps2c", bufs=3, space="PSUM") as ps2cpool,\
                 tc.tile_pool(name="ps2r", bufs=2, space="PSUM") as ps2rpool:
                for p in range(8):
                    a, b = 2 * p, 2 * p + 1
                    cc = p * 256
                    pA = ps2cpool.tile([128, 512], F32, tag="p2")
                    mm(pA[:, 0:256], z2p[p][:, 0:128], 'M2_lo', True, True)
                    mm(pA[:, 256:512], z2p[p][:, 128:256], 'M2_lo', True, True)
                    pB = ps2cpool.tile([128, 512], F32, tag="p2")
                    lp = lh2all[:, cc:cc + 256]
                    mm(pB[:], lp[:, 0::2], 'L2_hi_R', True, False)
                    mm(pB[:], lp[:, 1::2], 'L2_hi_I', False, True)
                    pC = ps2cpool.tile([128, 512], F32, tag="p2")
                    hp = hl2all[:, cc:cc + 256]
                    hq = hh2all[:, cc:cc + 256]
                    mm(pC[:], hp[:, 0::2], 'L2_lo_R', True, False)
                    mm(pC[:], hp[:, 1::2], 'L2_lo_I', False, False)
                    mm(pC[:], hq[:, 0::2], 'L2_hi_R', False, False)
                    mm(pC[:], hq[:, 1::2], 'L2_hi_I', False, True)
                    y1zT_s = midpool.tile([128, 512], F16, tag="y1zT2")
                    nc.scalar.copy(y1zT_s[:], pA[:])
                    b1_s = midpool.tile([128, 512], F16, tag="b1_2")
                    nc.vector.tensor_copy(out=b1_s[:], in_=pB[:])
                    b2_s = midpool.tile([128, 512], F16, tag="b2_2")
                    nc.vector.tensor_copy(out=b2_s[:], in_=pC[:])

                    for i, img in enumerate((a, b)):
                        off = i * 64
                        abase = i * 256
                        p2r = ps2rpool.tile([128, 512], F32, tag="p2r")
                        for m in range(2):
                            zc = p2r[:, m * 256:(m + 1) * 256]
                            msl = slice(m * 128, (m + 1) * 128)
                            osl = slice(256 + m * 128, 256 + (m + 1) * 128)
                            mm(zc, y1zT_s[:, abase + m * 128:
                                          abase + (m + 1) * 128],
                               'M2_lo', True, False)
                            mm(zc, b1_s[off:off + 64, msl], 'Be2_lo',
                               False, False, poff=off)
                            mm(zc, b1_s[off:off + 64, osl], 'Bo2_lo',
                               False, False, poff=off)
                            mm(zc, b2_s[off:off + 64, msl], 'Be2_hi',
                               False, False, poff=off)
                            mm(zc, b2_s[off:off + 64, osl], 'Bo2_hi',
                               False, True, poff=off)
                            zdst = z1s_ap(img)
                            if m == 0:
                                nc.scalar.copy(zdst[:, 0:256], zc)
                            else:
                                nc.vector.tensor_copy(
                                    out=zdst[:, 256:512], in_=zc)

            # ===========================================================
            # Phase L1: z1 [256,256] + yh0 bands -> out [256,256], all imgs
            # ===========================================================
            with tc.tile_pool(name="ps1c", bufs=3, space="PSUM") as ps1cpool,\
                 tc.tile_pool(name="ps1r", bufs=2, space="PSUM") as ps1rpool:
                for img in range(IMGS_PER_CORE):
                    yh0t = yh0g[img // 4]
                    ib = (img % 4) * 1536
                    o_t = {o: yh0t[:, ib + o * 256:ib + (o + 1) * 256]
                           for o in range(6)}
                    z1_s = z1s_ap(img)
                    # phase A: y1 = band + lowpass, merged in w-polyphase
                    # layout [E(h 256) | O(h 256)]  (partitions = w')
                    p1a = ps1cpool.tile([128, 512], F32, tag="p1")
                    y1_p = p1a[:]
                    mm(y1_p, o_t[0][:, 0::2], 'L1hi_w1r', True, False)
                    mm(y1_p, o_t[5][:, 0::2], 'L1hi_w2r', False, False)
                    mm(y1_p, o_t[0][:, 1::2], 'L1hi_w1i', False, False)
                    mm(y1_p, o_t[5][:, 1::2], 'L1hi_w2i', False, False)
                    mm(p1a[:, 0:256], z1_s[:, 0:256:2], 'Alo_a',
                       False, False)
                    mm(p1a[:, 0:256], z1_s[:, 256:512:2], 'Alo_b',
                       False, True)
                    mm(p1a[:, 256:512], z1_s[:, 1:256:2], 'Alo_a',
                       False, False)
                    mm(p1a[:, 256:512], z1_s[:, 257:512:2], 'Alo_b',
                       False, True)
                    y1_s = midpool.tile([128, 512], F16, tag="y1m")
                    nc.vector.tensor_copy(out=y1_s[:], in_=y1_p)

                    # phase B: y2b e|o [0:512)
                    p1b = ps1cpool.tile([128, 512], F32, tag="p1")
                    y2b_p = p1b[:]
                    mm(y2b_p, o_t[2][:, 0::2], 'L1lo_w1r', True, False)
                    mm(y2b_p, o_t[3][:, 0::2], 'L1lo_w2r', False, False)
                    mm(y2b_p, o_t[2][:, 1::2], 'L1lo_w1i', False, False)
                    mm(y2b_p, o_t[3][:, 1::2], 'L1lo_w2i', False, False)
                    mm(y2b_p, o_t[1][:, 0::2], 'L1hi_w1r', False, False)
                    mm(y2b_p, o_t[4][:, 0::2], 'L1hi_w2r', False, False)
                    mm(y2b_p, o_t[1][:, 1::2], 'L1hi_w1i', False, False)
                    mm(y2b_p, o_t[4][:, 1::2], 'L1hi_w2i', False, True)
                    y2b1_s = midpool.tile([128, 512], F16, tag="y2b1")
                    nc.vector.tensor_copy(out=y2b1_s[:], in_=y2b_p)

                    # row stage -> out [256, 256] in two h-chunks; single
                    # store DMA per image ([a p] x <- p [a x])
                    p1r = ps1rpool.tile([128, 512], F32, tag="p1r")
                    ot = outpool.tile([128, 512], F16, tag="ot")
                    for m in range(2):
                        oc = p1r[:, m * 256:(m + 1) * 256]
                        msl = slice(m * 128, (m + 1) * 128)
                        osl = slice(256 + m * 128, 256 + (m + 1) * 128)
                        mm(oc, y1_s[:, msl], 'Be1_lo', True, False)
                        mm(oc, y1_s[:, osl], 'Bo1_lo', False, False)
                        mm(oc, y2b1_s[:, msl], 'Be1_hi', False, False)
                        mm(oc, y2b1_s[:, osl], 'Bo1_hi', False, True)
                        if m == 0:
                            nc.scalar.copy(ot[:, 0:256], oc)
                        else:
                            nc.vector.tensor_copy(out=ot[:, 256:512], in_=oc)
                    nc.sync.dma_start(
                        out=out_d[img].rearrange("(a p) x -> p a x", a=2),
                        in_=ot.rearrange("p (a x) -> p a x", a=2))

    split_excess_waits(nc)
    return nc


# ---------------------------------------------------------------------------
# Entry point
# ---------------------------------------------------------------------------
_NC_CACHE = []
_LAST_RESULT = []  # last BassKernelResults (exec_time_ns when BASS_TRACE=1)


def _axon_reset():
    try:
        import ctypes
        lib = ctypes.CDLL('/opt/axon/libaxon_pjrt.so')
        lib.axon_reset.restype = ctypes.c_int64
        lib.axon_reset()
    except Exception:
        pass


def kernel(yl, yh0, yh1, yh2, g0o, g1o, g0a, g0b, g1a, g1b):
    yl = np.ascontiguousarray(np.asarray(yl, np.float16))
    yh0 = np.ascontiguousarray(np.asarray(yh0, np.float16))
    yh1 = np.ascontiguousarray(np.asarray(yh1, np.float16))
    yh2 = np.ascontiguousarray(np.asarray(yh2, np.float16))
    assert yl.shape == (8, 16, 64, 64)

    mats = build_matrices(g0o, g1o, g0a, g0b, g1a, g1b)
    blobA0, blobA1, blobB = pack_blobs(mats)
    if not _NC_CACHE:
        _NC_CACHE.append(build_nc())
    nc = _NC_CACHE[0]

    in_maps = []
    for core in range(N_CORES):
        m = {"yl": yl[core], "yh0": yh0[core],
             "yh1": yh1[core], "yh2": yh2[core],
             "matsA0": blobA0, "matsA1": blobA1, "matsB": blobB}
        in_maps.append(m)

    try:
        res = run_bass_kernel_spmd(nc, in_maps, list(range(N_CORES)))
    except Exception as e:  # wedged exec unit: reset the axon device, retry
        if "UNAVAILABLE" not in str(e) and "unrecoverable" not in str(e):
            raise
        _axon_reset()
        res = run_bass_kernel_spmd(nc, in_maps, list(range(N_CORES)))
    _LAST_RESULT.clear()
    _LAST_RESULT.append(res)
    out = np.stack([res.results[i]["out"] for i in range(N_CORES)], axis=0)
    return np.ascontiguousarray(out.astype(np.float32))
